# revision 11
# baseline (speedup 1.0000x reference)
"""Trainium2 Bass kernel for ConcatVolume (stereo cost-volume concat).

Reference semantics (B=1, F=32, H=128, W=256, D=48, bins = arange(48)):
  vol_lr[0, 0:F,  d, h, w] = fl[0,:,h,w]        if w >= d      else 0
  vol_lr[0, F:2F, d, h, w] = fr[0,:,h,w-d]      if w >= d      else 0
  vol_rl[0, 0:F,  d, h, w] = fl[0,:,h,w+d]      if w <  W-d    else 0
  vol_rl[0, F:2F, d, h, w] = fr[0,:,h,w]        if w <  W-d    else 0
Returns (vol_lr, vol_rl), each [1, 2F, D, H, W] f32 (~403 MB each).

Strategy: the whole problem is data movement (memory-bound). Shard the D
axis across the 8 cores (6 bins/core). To keep a single SPMD program with
compile-time access patterns, the host passes each core *windowed* views of
zero-padded inputs so that per-(local bin j) source offsets are static:

  flp  = (fl ++ 48 zero cols)[base : base+261]   -> rl-left  src col = w + j
  frp  = (48 zero cols ++ fr)[43-base : 304-base]-> lr-right src col = w + 5 - j
  fl48 = fl[:, :, 48:]  (mask w>=d always true there)    -> lr-left cols 48..255
  fr208= fr[:, :, :208] (mask w<W-d always true there)   -> rl-right cols 0..207
  p1[j] = fl[:, :, :48]  * (w >= d)   boundary strip, host-masked
  p2[j] = fr[:, :, 208:] * (w < W-d)  boundary strip, host-masked

Device work: stage the four reused tensors in SBUF once (~15 MB), then per
local bin j issue six DMA stores (4 big SBUF->DRAM shifted copies + 2 small
DRAM->DRAM boundary strips). Everything is DMA; no compute engines.
"""

import numpy as np

B, F, H, W, D = 1, 32, 128, 256, 48
NCORES = 8
DPC = D // NCORES  # 6 bins per core (D-sharded variants)
PAD = 48  # > max disparity (47)
WIN = W + DPC - 1  # 261: window width covering all 6 shifts
HPC = H // NCORES  # 16 rows per core (H-sharded variant)
HH, HL = 4, 4  # h = (h_hi, h_lo); partition = f*HH + h_hi
WP = W + PAD  # 304: padded width for shifted windows

_cache = {}


VARIANT = "H3"


def _build_program(reps=1, variant=None, loop_reps=1, loads_in_loop=False):
    v = variant or VARIANT
    if v == "B":
        return _build_program_b(reps)
    if v == "C":
        return _build_program_c(reps, loop_reps)
    if v == "D":
        return _build_program_d(loop_reps, loads_in_loop)
    if v == "H":
        return _build_program_h(loop_reps, loads_in_loop)
    if v == "H2":
        return _build_program_h2(loop_reps, loads_in_loop)
    if v == "H3":
        return _build_program_h3(loop_reps, loads_in_loop)
    return _build_program_a(reps, loop_reps, loads_in_loop)


def _build_program_h3(loop_reps=1, loads_in_loop=False):
    """Variant H3 = H2 with the DVE->store dependency stalls removed.

    probe2 showed H2's exact store schedule with no DVE deps runs at
    241 us (sync+scalar) vs H2's 309 us, so the whole gap was stores
    waiting on just-in-time DVE output (2-deep buffering) plus ~6 us of
    SWDGE drag.  Changes vs H2:
      - quadrant tiles are 4-deep buffered: DVE materializes bin d+3
        while bin d stores, so stores never wait on DVE;
      - all 192 stores go on the two HWDGE queues (sync/scalar);
        gpsimd only does the input loads;
      - derive() (bodies/strips) and the 96 mask tiles are hoisted out
        of the timing loop (they are setup work in the real kernel);
        only the flp/frp input loads stay per-iteration.
    """
    import contextlib

    import concourse.bacc as bacc
    import concourse.mybir as mybir
    import concourse.tile as tile

    nc = bacc.Bacc(
        "TRN2",
        target_bir_lowering=False,
        debug=False,
        enable_asserts=False,
        num_devices=NCORES,
    )

    f32 = mybir.dt.float32
    P = F * HH  # 128 partitions
    NB = 4  # quadrant-tile buffer depth
    flp = nc.dram_tensor("flp", [P, HL * WP], f32, kind="ExternalInput").ap()
    frp = nc.dram_tensor("frp", [P, HL * WP], f32, kind="ExternalInput").ap()
    wid = nc.dram_tensor("wid", [P, HL * PAD], f32, kind="ExternalInput").ap()
    olr = nc.dram_tensor("olr", [2 * F, D, HPC, W], f32, kind="ExternalOutput").ap()
    orl = nc.dram_tensor("orl", [2 * F, D, HPC, W], f32, kind="ExternalOutput").ap()

    with tile.TileContext(nc) as tc:
        with tc.tile_pool(name="stage", bufs=1) as pool:
            s_flp = pool.tile([P, HL * WP], f32, tag="s_flp")
            s_frp = pool.tile([P, HL * WP], f32, tag="s_frp")
            s_wid = pool.tile([P, HL * PAD], f32, tag="s_wid")
            s_flpre = pool.tile([P, HL * PAD], f32, tag="s_flpre")
            s_frsuf = pool.tile([P, HL * PAD], f32, tag="s_frsuf")

            def tilesN(nm):
                return [
                    pool.tile(
                        [P, HL * W], f32, name=f"{nm}{i}", tag=f"{nm}{i}"
                    )
                    for i in range(NB)
                ]

            s_fla = tilesN("s_fla")  # unshifted fl (strip mutable)
            s_frb = tilesN("s_frb")  # unshifted fr (strip mutable)
            s_shl = tilesN("s_shl")  # rl-left shifted fl, materialized
            s_shr = tilesN("s_shr")  # lr-right shifted fr, materialized
            m_ge = [
                pool.tile(
                    [P, HL * PAD], f32, name=f"m_ge{d}", tag=f"m_ge{d}"
                )
                for d in range(D)
            ]
            m_lt = [
                pool.tile(
                    [P, HL * PAD], f32, name=f"m_lt{d}", tag=f"m_lt{d}"
                )
                for d in range(D)
            ]

            def v3(t):
                return t[:].rearrange("p (b w) -> p b w", b=HL)

            def dr4(slab):
                return slab.rearrange("f (a b) w -> f a b w", a=HH)

            v_flp = v3(s_flp)
            v_frp = v3(s_frp)

            # one-time setup: wid ramp + 96 strip-mask tiles
            nc.sync.dma_start(s_wid[:], wid)
            for d in range(D):
                nc.vector.tensor_scalar(
                    m_ge[d][:], s_wid[:], float(d), None,
                    mybir.AluOpType.is_ge,
                )
                nc.vector.tensor_scalar(
                    m_lt[d][:], s_wid[:], float(PAD - d), None,
                    mybir.AluOpType.is_lt,
                )

            def do_loads():
                nc.gpsimd.dma_start(s_flp[:], flp)
                nc.gpsimd.dma_start(s_frp[:], frp)

            # one-time setup: initial loads + derived bodies/strips
            do_loads()
            nc.vector.tensor_copy(v3(s_flpre), v_flp[:, :, 0:PAD])
            nc.vector.tensor_copy(v3(s_frsuf), v_frp[:, :, W:WP])
            for i in range(NB):
                nc.vector.tensor_copy(
                    v3(s_fla[i])[:, :, PAD:W], v_flp[:, :, PAD:W]
                )
                nc.vector.tensor_copy(
                    v3(s_frb[i])[:, :, 0 : W - PAD], v_frp[:, :, PAD:W]
                )

            engines = [nc.sync, nc.scalar]

            loop_cm = (
                tc.For_i(0, loop_reps, 1)
                if loop_reps > 1
                else contextlib.nullcontext()
            )
            with loop_cm:
                if loads_in_loop:
                    do_loads()
                for d in range(D):
                    i2 = d % NB
                    # materialize the two shifted windows contiguously
                    nc.vector.tensor_copy(
                        v3(s_shl[i2]), v_flp[:, :, d : d + W]
                    )
                    nc.vector.tensor_copy(
                        v3(s_shr[i2]), v_frp[:, :, PAD - d : PAD - d + W]
                    )
                    # masked boundary strips in place
                    nc.vector.tensor_mul(
                        v3(s_fla[i2])[:, :, 0:PAD], v3(s_flpre), v3(m_ge[d])
                    )
                    nc.vector.tensor_mul(
                        v3(s_frb[i2])[:, :, W - PAD : W],
                        v3(s_frsuf), v3(m_lt[d]),
                    )
                    e = [engines[(4 * d + k) % 2] for k in range(4)]
                    e[0].dma_start(dr4(olr[0:F, d, :, :]), s_fla[i2][:])
                    e[1].dma_start(dr4(olr[F : 2 * F, d, :, :]), s_shr[i2][:])
                    e[2].dma_start(dr4(orl[0:F, d, :, :]), s_shl[i2][:])
                    e[3].dma_start(dr4(orl[F : 2 * F, d, :, :]), s_frb[i2][:])

    nc.compile()
    return nc


def _build_program_h2(loop_reps=1, loads_in_loop=False):
    """Variant H2 = H with every store shipping 4KB DMA descriptors
    (empirically the per-core DMA sweet spot: ~413 GB/s vs 339 GB/s for
    the 1KB-run window stores of variant H):

    - shifted windows are DVE-materialized per bin into contiguous
      double-buffered tiles before storing (DVE has ~3x slack vs DMA);
    - boundary strips are DVE-masked in place using 96 precomputed
      (hoisted) mask tiles;
    - only flp/frp/wid (~1.35 MB/core) are read from DRAM; unshifted
      bodies and pristine strips are DVE-derived from the padded windows.
    """
    import contextlib

    import concourse.bacc as bacc
    import concourse.mybir as mybir
    import concourse.tile as tile

    nc = bacc.Bacc(
        "TRN2",
        target_bir_lowering=False,
        debug=False,
        enable_asserts=False,
        num_devices=NCORES,
    )

    f32 = mybir.dt.float32
    P = F * HH  # 128 partitions
    flp = nc.dram_tensor("flp", [P, HL * WP], f32, kind="ExternalInput").ap()
    frp = nc.dram_tensor("frp", [P, HL * WP], f32, kind="ExternalInput").ap()
    wid = nc.dram_tensor("wid", [P, HL * PAD], f32, kind="ExternalInput").ap()
    olr = nc.dram_tensor("olr", [2 * F, D, HPC, W], f32, kind="ExternalOutput").ap()
    orl = nc.dram_tensor("orl", [2 * F, D, HPC, W], f32, kind="ExternalOutput").ap()

    with tile.TileContext(nc) as tc:
        with tc.tile_pool(name="stage", bufs=1) as pool:
            s_flp = pool.tile([P, HL * WP], f32, tag="s_flp")
            s_frp = pool.tile([P, HL * WP], f32, tag="s_frp")
            s_wid = pool.tile([P, HL * PAD], f32, tag="s_wid")
            s_flpre = pool.tile([P, HL * PAD], f32, tag="s_flpre")
            s_frsuf = pool.tile([P, HL * PAD], f32, tag="s_frsuf")

            def tiles2(nm, width):
                return [
                    pool.tile(
                        [P, HL * width], f32, name=f"{nm}{i}", tag=f"{nm}{i}"
                    )
                    for i in range(2)
                ]

            s_fla = tiles2("s_fla", W)  # unshifted fl (strip mutable)
            s_frb = tiles2("s_frb", W)  # unshifted fr (strip mutable)
            s_shl = tiles2("s_shl", W)  # rl-left shifted fl, materialized
            s_shr = tiles2("s_shr", W)  # lr-right shifted fr, materialized
            m_ge = [
                pool.tile(
                    [P, HL * PAD], f32, name=f"m_ge{d}", tag=f"m_ge{d}"
                )
                for d in range(D)
            ]
            m_lt = [
                pool.tile(
                    [P, HL * PAD], f32, name=f"m_lt{d}", tag=f"m_lt{d}"
                )
                for d in range(D)
            ]

            def v3(t):
                return t[:].rearrange("p (b w) -> p b w", b=HL)

            def dr4(slab):
                return slab.rearrange("f (a b) w -> f a b w", a=HH)

            v_flp = v3(s_flp)
            v_frp = v3(s_frp)

            # one-time constants: wid ramp + 96 strip-mask tiles
            nc.sync.dma_start(s_wid[:], wid)
            for d in range(D):
                nc.vector.tensor_scalar(
                    m_ge[d][:], s_wid[:], float(d), None,
                    mybir.AluOpType.is_ge,
                )
                nc.vector.tensor_scalar(
                    m_lt[d][:], s_wid[:], float(PAD - d), None,
                    mybir.AluOpType.is_lt,
                )

            def do_loads():
                nc.sync.dma_start(s_flp[:], flp)
                nc.scalar.dma_start(s_frp[:], frp)

            def derive():
                # pristine strips + immutable bodies from the padded windows
                nc.vector.tensor_copy(v3(s_flpre), v_flp[:, :, 0:PAD])
                nc.vector.tensor_copy(v3(s_frsuf), v_frp[:, :, W:WP])
                for i in range(2):
                    nc.vector.tensor_copy(
                        v3(s_fla[i])[:, :, PAD:W], v_flp[:, :, PAD:W]
                    )
                    nc.vector.tensor_copy(
                        v3(s_frb[i])[:, :, 0 : W - PAD], v_frp[:, :, PAD:W]
                    )

            if not loads_in_loop:
                do_loads()
                derive()

            engines = [nc.sync, nc.scalar, nc.gpsimd]

            loop_cm = (
                tc.For_i(0, loop_reps, 1)
                if loop_reps > 1
                else contextlib.nullcontext()
            )
            with loop_cm:
                if loads_in_loop:
                    do_loads()
                    derive()
                for d in range(D):
                    i2 = d % 2
                    # materialize the two shifted windows contiguously
                    nc.vector.tensor_copy(
                        v3(s_shl[i2]), v_flp[:, :, d : d + W]
                    )
                    nc.vector.tensor_copy(
                        v3(s_shr[i2]), v_frp[:, :, PAD - d : PAD - d + W]
                    )
                    # masked boundary strips in place
                    nc.vector.tensor_mul(
                        v3(s_fla[i2])[:, :, 0:PAD], v3(s_flpre), v3(m_ge[d])
                    )
                    nc.vector.tensor_mul(
                        v3(s_frb[i2])[:, :, W - PAD : W],
                        v3(s_frsuf), v3(m_lt[d]),
                    )
                    e = [engines[(4 * d + k) % 3] for k in range(4)]
                    e[0].dma_start(dr4(olr[0:F, d, :, :]), s_fla[i2][:])
                    e[1].dma_start(dr4(olr[F : 2 * F, d, :, :]), s_shr[i2][:])
                    e[2].dma_start(dr4(orl[0:F, d, :, :]), s_shl[i2][:])
                    e[3].dma_start(dr4(orl[F : 2 * F, d, :, :]), s_frb[i2][:])

    nc.compile()
    return nc


def _build_program_h(loop_reps=1, loads_in_loop=False):
    """Variant H: shard the H axis (16 rows/core, all 48 bins/core).

    Per-core reads drop to ~3.3 MB (vs ~10.9 MB for D-sharding) and every
    output byte ships in a full-width store whose contiguous runs are
    >= 1 KB (full DMA bandwidth; no <512B boundary-strip stores).

    SBUF layout: partition p = f*4 + h_hi (128), free = (h_lo, w).
    Per bin d:
      lr-left  = fl, cols < d zeroed  -> masked in-place into a
                 double-buffered full-width tile (DVE), stored full-width
      lr-right = fr shifted by +d     -> window of zero-padded fr (no mask)
      rl-left  = fl shifted by -d     -> window of zero-padded fl (no mask)
      rl-right = fr, cols >= W-d zeroed -> masked in-place, stored full-width
    """
    import contextlib

    import concourse.bacc as bacc
    import concourse.mybir as mybir
    import concourse.tile as tile

    nc = bacc.Bacc(
        "TRN2",
        target_bir_lowering=False,
        debug=False,
        enable_asserts=False,
        num_devices=NCORES,
    )

    f32 = mybir.dt.float32
    P = F * HH  # 128 partitions
    fla = nc.dram_tensor("fla", [P, HL * W], f32, kind="ExternalInput").ap()
    frb = nc.dram_tensor("frb", [P, HL * W], f32, kind="ExternalInput").ap()
    flp = nc.dram_tensor("flp", [P, HL * WP], f32, kind="ExternalInput").ap()
    frp = nc.dram_tensor("frp", [P, HL * WP], f32, kind="ExternalInput").ap()
    flpre = nc.dram_tensor("flpre", [P, HL * PAD], f32, kind="ExternalInput").ap()
    frsuf = nc.dram_tensor("frsuf", [P, HL * PAD], f32, kind="ExternalInput").ap()
    wid = nc.dram_tensor("wid", [P, HL * PAD], f32, kind="ExternalInput").ap()
    olr = nc.dram_tensor("olr", [2 * F, D, HPC, W], f32, kind="ExternalOutput").ap()
    orl = nc.dram_tensor("orl", [2 * F, D, HPC, W], f32, kind="ExternalOutput").ap()

    with tile.TileContext(nc) as tc:
        with (
            tc.tile_pool(name="stage", bufs=1) as pool,
            tc.tile_pool(name="mpool", bufs=4) as mpool,
        ):
            s_fla = [
                pool.tile(
                    [P, HL * W], f32, name=f"s_fla{i}", tag=f"s_fla{i}"
                )
                for i in range(2)
            ]
            s_frb = [
                pool.tile(
                    [P, HL * W], f32, name=f"s_frb{i}", tag=f"s_frb{i}"
                )
                for i in range(2)
            ]
            s_flp = pool.tile([P, HL * WP], f32, tag="s_flp")
            s_frp = pool.tile([P, HL * WP], f32, tag="s_frp")
            s_flpre = pool.tile([P, HL * PAD], f32, tag="s_flpre")
            s_frsuf = pool.tile([P, HL * PAD], f32, tag="s_frsuf")
            s_wid = pool.tile([P, HL * PAD], f32, tag="s_wid")

            def do_loads():
                nc.sync.dma_start(s_fla[0][:], fla)
                nc.scalar.dma_start(s_frb[0][:], frb)
                nc.gpsimd.dma_start(s_flp[:], flp)
                nc.sync.dma_start(s_frp[:], frp)
                nc.scalar.dma_start(s_fla[1][:], fla)
                nc.gpsimd.dma_start(s_frb[1][:], frb)
                nc.sync.dma_start(s_flpre[:], flpre)
                nc.scalar.dma_start(s_frsuf[:], frsuf)
                nc.gpsimd.dma_start(s_wid[:], wid)

            if not loads_in_loop:
                do_loads()

            # 3D SBUF views [p, hl, w] (partition dim intact for DVE)
            def v3(t, width):
                return t[:].rearrange("p (b w) -> p b w", b=HL)

            # 4D DRAM views [f, hh, hl, w] matching partition-major order
            def dr4(slab):
                return slab.rearrange("f (a b) w -> f a b w", a=HH)

            v_flp = v3(s_flp, WP)
            v_frp = v3(s_frp, WP)
            v_flpre = v3(s_flpre, PAD)
            v_frsuf = v3(s_frsuf, PAD)
            v_wid = v3(s_wid, PAD)

            engines = [nc.sync, nc.scalar, nc.gpsimd]

            loop_cm = (
                tc.For_i(0, loop_reps, 1)
                if loop_reps > 1
                else contextlib.nullcontext()
            )
            with loop_cm:
                if loads_in_loop:
                    do_loads()
                for d in range(D):
                    ia = d % 2
                    if d > 0:
                        # lr-left strip: keep fl col w (w<48) iff w >= d
                        m1 = mpool.tile([P, HL * PAD], f32, tag="m1")
                        nc.vector.tensor_scalar(
                            m1[:], s_wid[:], float(d), None,
                            mybir.AluOpType.is_ge,
                        )
                        nc.vector.tensor_mul(
                            v3(s_fla[ia], W)[:, :, 0:PAD],
                            v_flpre, v3(m1, PAD),
                        )
                        # rl-right strip: keep fr col 208+k iff k < 48-d
                        m2 = mpool.tile([P, HL * PAD], f32, tag="m2")
                        nc.vector.tensor_scalar(
                            m2[:], s_wid[:], float(PAD - d), None,
                            mybir.AluOpType.is_lt,
                        )
                        nc.vector.tensor_mul(
                            v3(s_frb[ia], W)[:, :, W - PAD : W],
                            v_frsuf, v3(m2, PAD),
                        )
                    e = [engines[(4 * d + k) % 3] for k in range(4)]
                    # lr-left: full-width masked fl
                    e[0].dma_start(dr4(olr[0:F, d, :, :]), s_fla[ia][:])
                    # lr-right: fr shifted +d (window of padded fr)
                    e[1].dma_start(
                        dr4(olr[F : 2 * F, d, :, :]),
                        v_frp[:, :, PAD - d : PAD - d + W],
                    )
                    # rl-left: fl shifted -d (window of padded fl)
                    e[2].dma_start(
                        dr4(orl[0:F, d, :, :]), v_flp[:, :, d : d + W]
                    )
                    # rl-right: full-width masked fr
                    e[3].dma_start(dr4(orl[F : 2 * F, d, :, :]), s_frb[ia][:])

    nc.compile()
    return nc


def _build_program_d(loop_reps=1, loads_in_loop=False):
    """Variant D = A with reduced HBM reads:
    - the unshifted quadrants read the padded windows at a per-core dynamic
      offset (48-6c / 5+6c via partition_id) instead of separate fl48/fr208
      inputs (-6.5 MB/core of loads);
    - boundary strips are masked on-device with DVE (wid >= d / wid < 48-d,
      thresholds passed per-core as a tiny SBUF scalar input) instead of
      host-precomputed p1/p2 strips (-9.4 MB/core of DRAM reads)."""
    import contextlib

    import concourse.bacc as bacc
    import concourse.bass as bass
    import concourse.mybir as mybir
    import concourse.tile as tile

    nc = bacc.Bacc(
        "TRN2",
        target_bir_lowering=False,
        debug=False,
        enable_asserts=False,
        num_devices=NCORES,
    )

    f32 = mybir.dt.float32
    flp = nc.dram_tensor("flp", [H, F * WIN], f32, kind="ExternalInput").ap()
    frp = nc.dram_tensor("frp", [H, F * WIN], f32, kind="ExternalInput").ap()
    flpre = nc.dram_tensor("flpre", [H, F * PAD], f32, kind="ExternalInput").ap()
    frsuf = nc.dram_tensor("frsuf", [H, F * PAD], f32, kind="ExternalInput").ap()
    wid = nc.dram_tensor("wid", [H, F * PAD], f32, kind="ExternalInput").ap()
    thr = nc.dram_tensor("thr", [H, 2 * DPC], f32, kind="ExternalInput").ap()
    olr = nc.dram_tensor("olr", [2 * F, DPC, H, W], f32, kind="ExternalOutput").ap()
    orl = nc.dram_tensor("orl", [2 * F, DPC, H, W], f32, kind="ExternalOutput").ap()

    with tile.TileContext(nc) as tc:
        with (
            tc.tile_pool(name="stage", bufs=1) as pool,
            tc.tile_pool(name="mpool", bufs=3) as mpool,
        ):
            s_flp = pool.tile([H, F * WIN], f32, tag="s_flp")
            s_frp = pool.tile([H, F * WIN], f32, tag="s_frp")
            s_flpre = pool.tile([H, F * PAD], f32, tag="s_flpre")
            s_frsuf = pool.tile([H, F * PAD], f32, tag="s_frsuf")
            s_wid = pool.tile([H, F * PAD], f32, tag="s_wid")
            s_thr = pool.tile([H, 2 * DPC], f32, tag="s_thr")

            def do_loads():
                nc.sync.dma_start(s_flp[:], flp)
                nc.scalar.dma_start(s_frp[:], frp)
                nc.sync.dma_start(s_flpre[:], flpre)
                nc.scalar.dma_start(s_frsuf[:], frsuf)
                nc.sync.dma_start(s_wid[:], wid)
                nc.scalar.dma_start(s_thr[:], thr)

            if not loads_in_loop:
                do_loads()

            v_flp = s_flp[:].rearrange("h (f w) -> h f w", f=F)
            v_frp = s_frp[:].rearrange("h (f w) -> h f w", f=F)

            def hfw(dram_slab):
                return dram_slab.transpose([1, 0, 2])

            loop_cm = (
                tc.For_i(0, loop_reps, 1)
                if loop_reps > 1
                else contextlib.nullcontext()
            )
            with loop_cm:
                if loads_in_loop:
                    do_loads()
                pid_sp = nc.sync.partition_id()
                pid_act = nc.scalar.partition_id()
                off1 = PAD - pid_sp * DPC  # 48 - 6c: fl[w]=flp[w - 6c], w>=48
                off2 = (
                    DPC - 1 + pid_act * DPC
                )  # 5 + 6c: fr[w]=frp[w + 5 + 6c]
                for j in range(DPC):
                    # strips: mask on device, store via gpsimd
                    mask = mpool.tile([H, F * PAD], f32, tag="mask")
                    nc.vector.tensor_scalar(
                        mask[:],
                        s_wid[:],
                        s_thr[:, j : j + 1],
                        None,
                        mybir.AluOpType.is_ge,
                    )
                    m1 = mpool.tile([H, F * PAD], f32, tag="m1")
                    nc.vector.tensor_mul(m1[:], s_flpre[:], mask[:])
                    nc.gpsimd.dma_start(
                        hfw(olr[0:F, j, :, 0:PAD]),
                        m1[:].rearrange("h (f w) -> h f w", f=F),
                    )
                    mask2 = mpool.tile([H, F * PAD], f32, tag="mask2")
                    nc.vector.tensor_scalar(
                        mask2[:],
                        s_wid[:],
                        s_thr[:, DPC + j : DPC + j + 1],
                        None,
                        mybir.AluOpType.is_lt,
                    )
                    m2 = mpool.tile([H, F * PAD], f32, tag="m2")
                    nc.vector.tensor_mul(m2[:], s_frsuf[:], mask2[:])
                    nc.gpsimd.dma_start(
                        hfw(orl[F : 2 * F, j, :, W - PAD : W]),
                        m2[:].rearrange("h (f w) -> h f w", f=F),
                    )
                    # lr-left cols 48..: dynamic window of flp
                    nc.sync.dma_start(
                        hfw(olr[0:F, j, :, PAD:W]),
                        v_flp[:, :, bass.ds(off1, W - PAD)],
                    )
                    # lr-right: shifted window of padded fr (static)
                    nc.scalar.dma_start(
                        hfw(olr[F : 2 * F, j, :, :]),
                        v_frp[:, :, DPC - 1 - j : DPC - 1 - j + W],
                    )
                    # rl-left: shifted window of padded fl (static)
                    nc.sync.dma_start(
                        hfw(orl[0:F, j, :, :]), v_flp[:, :, j : j + W]
                    )
                    # rl-right cols 0..207: dynamic window of frp
                    nc.scalar.dma_start(
                        hfw(orl[F : 2 * F, j, :, 0 : W - PAD]),
                        v_frp[:, :, bass.ds(off2, W - PAD)],
                    )

    nc.compile()
    return nc


def _build_program_a(reps=1, loop_reps=1, loads_in_loop=False):
    import concourse.bacc as bacc
    import concourse.mybir as mybir
    import concourse.tile as tile

    nc = bacc.Bacc(
        "TRN2",
        target_bir_lowering=False,
        debug=False,
        enable_asserts=False,
        num_devices=NCORES,
    )

    f32 = mybir.dt.float32
    # staging inputs come in SBUF-friendly layout [H, F*width] (host transposes)
    flp = nc.dram_tensor("flp", [H, F * WIN], f32, kind="ExternalInput").ap()
    frp = nc.dram_tensor("frp", [H, F * WIN], f32, kind="ExternalInput").ap()
    fl48 = nc.dram_tensor("fl48", [H, F * (W - PAD)], f32, kind="ExternalInput").ap()
    fr208 = nc.dram_tensor("fr208", [H, F * (W - PAD)], f32, kind="ExternalInput").ap()
    p1 = nc.dram_tensor("p1", [DPC, F, H, PAD], f32, kind="ExternalInput").ap()
    p2 = nc.dram_tensor("p2", [DPC, F, H, PAD], f32, kind="ExternalInput").ap()
    olr = nc.dram_tensor("olr", [2 * F, DPC, H, W], f32, kind="ExternalOutput").ap()
    orl = nc.dram_tensor("orl", [2 * F, DPC, H, W], f32, kind="ExternalOutput").ap()

    with tile.TileContext(nc) as tc:
        with tc.tile_pool(name="stage", bufs=1) as pool:
            # SBUF layout: partition = h (128), free = f*width + w
            s_flp = pool.tile([H, F * WIN], f32, tag="s_flp")
            s_frp = pool.tile([H, F * WIN], f32, tag="s_frp")
            s_fl48 = pool.tile([H, F * (W - PAD)], f32, tag="s_fl48")
            s_fr208 = pool.tile([H, F * (W - PAD)], f32, tag="s_fr208")

            def do_loads():
                nc.sync.dma_start(s_flp[:], flp)
                nc.scalar.dma_start(s_frp[:], frp)
                nc.sync.dma_start(s_fl48[:], fl48)
                nc.scalar.dma_start(s_fr208[:], fr208)

            if not loads_in_loop:
                do_loads()

            # SBUF views with partition (h) outermost: [h, f, w]
            v_flp = s_flp[:].rearrange("h (f w) -> h f w", f=F)
            v_frp = s_frp[:].rearrange("h (f w) -> h f w", f=F)
            v_fl48 = s_fl48[:].rearrange("h (f w) -> h f w", f=F)
            v_fr208 = s_fr208[:].rearrange("h (f w) -> h f w", f=F)

            def hfw(dram_slab):
                # DRAM slab [f, h, w] -> AP enumerated [h, f, w] to match SBUF
                return dram_slab.transpose([1, 0, 2])

            import contextlib

            loop_cm = (
                tc.For_i(0, loop_reps, 1)
                if loop_reps > 1
                else contextlib.nullcontext()
            )
            with loop_cm:
                if loads_in_loop:
                    do_loads()
                for _rep in range(reps):
                    for j in range(DPC):
                        # lr-left: cols 48.., strip covers 0..47
                        nc.sync.dma_start(hfw(olr[0:F, j, :, PAD:W]), v_fl48)
                        nc.gpsimd.dma_start(olr[0:F, j, :, 0:PAD], p1[j])
                        # lr-right: shifted window of padded fr
                        nc.scalar.dma_start(
                            hfw(olr[F : 2 * F, j, :, :]),
                            v_frp[:, :, DPC - 1 - j : DPC - 1 - j + W],
                        )
                        # rl-left: shifted window of padded fl
                        nc.sync.dma_start(
                            hfw(orl[0:F, j, :, :]), v_flp[:, :, j : j + W]
                        )
                        # rl-right: cols 0..207 from fr208, then strip p2[j]
                        nc.scalar.dma_start(
                            hfw(orl[F : 2 * F, j, :, 0 : W - PAD]), v_fr208
                        )
                        nc.gpsimd.dma_start(
                            orl[F : 2 * F, j, :, W - PAD : W], p2[j]
                        )

    nc.compile()
    return nc


def _build_program_b(reps=1):
    """Variant B: SBUF partitions = (f, h_hi) so DRAM-side store runs are
    8KB contiguous (vs 1KB in variant A). Full-width stores everywhere; the
    <=48-col boundary strips overwrite afterwards (WAW ordered by Tile)."""
    import concourse.bacc as bacc
    import concourse.mybir as mybir
    import concourse.tile as tile

    nc = bacc.Bacc(
        "TRN2",
        target_bir_lowering=False,
        debug=False,
        enable_asserts=False,
        num_devices=NCORES,
    )

    f32 = mybir.dt.float32
    HH, HL = 4, 32  # h = h_hi*HL + h_lo; partition = h_hi*F + f
    # staging inputs in [(HH*F), (HL*width)] layout (host packs)
    flp = nc.dram_tensor("flp", [HH * F, HL * WIN], f32, kind="ExternalInput").ap()
    frp = nc.dram_tensor("frp", [HH * F, HL * WIN], f32, kind="ExternalInput").ap()
    flf = nc.dram_tensor("flf", [HH * F, HL * W], f32, kind="ExternalInput").ap()
    frf = nc.dram_tensor("frf", [HH * F, HL * W], f32, kind="ExternalInput").ap()
    p1 = nc.dram_tensor("p1", [DPC, F, H, PAD], f32, kind="ExternalInput").ap()
    p2 = nc.dram_tensor("p2", [DPC, F, H, PAD], f32, kind="ExternalInput").ap()
    olr = nc.dram_tensor("olr", [2 * F, DPC, H, W], f32, kind="ExternalOutput").ap()
    orl = nc.dram_tensor("orl", [2 * F, DPC, H, W], f32, kind="ExternalOutput").ap()

    with tile.TileContext(nc) as tc:
        with tc.tile_pool(name="stage", bufs=1) as pool:
            s_flp = pool.tile([HH * F, HL * WIN], f32, tag="s_flp")
            s_frp = pool.tile([HH * F, HL * WIN], f32, tag="s_frp")
            s_flf = pool.tile([HH * F, HL * W], f32, tag="s_flf")
            s_frf = pool.tile([HH * F, HL * W], f32, tag="s_frf")

            nc.sync.dma_start(s_flp[:], flp)
            nc.scalar.dma_start(s_frp[:], frp)
            nc.sync.dma_start(s_flf[:], flf)
            nc.scalar.dma_start(s_frf[:], frf)

            # windowed views [h_hi, f, h_lo, w]
            v_flp = s_flp[:].rearrange("(a f) (b w) -> a f b w", f=F, b=HL)
            v_frp = s_frp[:].rearrange("(a f) (b w) -> a f b w", f=F, b=HL)

            for _rep in range(reps):
                for j in range(DPC):
                    # lr-left: full-width fl, strip overwrites cols 0..47
                    nc.sync.dma_start(
                        olr[0:F, j, :, :].rearrange("f (a b) w -> a f b w", a=HH),
                        s_flf[:],
                    )
                    nc.gpsimd.dma_start(olr[0:F, j, :, 0:PAD], p1[j])
                    # lr-right: shifted window of padded fr, per h_hi block
                    dst = olr[F : 2 * F, j, :, :].rearrange(
                        "f (a b) w -> a f b w", a=HH
                    )
                    s0 = DPC - 1 - j
                    for hh in range(HH):
                        nc.scalar.dma_start(
                            dst[hh], v_frp[hh, :, :, s0 : s0 + W]
                        )
                    # rl-left: shifted window of padded fl, per h_hi block
                    dst = orl[0:F, j, :, :].rearrange("f (a b) w -> a f b w", a=HH)
                    for hh in range(HH):
                        nc.sync.dma_start(dst[hh], v_flp[hh, :, :, j : j + W])
                    # rl-right: full-width fr, strip overwrites cols 208..255
                    nc.scalar.dma_start(
                        orl[F : 2 * F, j, :, :].rearrange(
                            "f (a b) w -> a f b w", a=HH
                        ),
                        s_frf[:],
                    )
                    nc.gpsimd.dma_start(orl[F : 2 * F, j, :, W - PAD : W], p2[j])

    nc.compile()
    return nc


def _build_program_c(reps=1, loop_reps=1):
    """Variant C: shifted stores as in A (partition=h, full 128-partition
    sources); the two unshifted full-width quadrants read (h_hi,f)-packed
    tiles so each is a single DMA with 8KB-contiguous DRAM runs, with the
    boundary strip overwriting afterwards."""
    import concourse.bacc as bacc
    import concourse.mybir as mybir
    import concourse.tile as tile

    nc = bacc.Bacc(
        "TRN2",
        target_bir_lowering=False,
        debug=False,
        enable_asserts=False,
        num_devices=NCORES,
    )

    f32 = mybir.dt.float32
    HH, HL = 4, 32
    flp = nc.dram_tensor("flp", [H, F * WIN], f32, kind="ExternalInput").ap()
    frp = nc.dram_tensor("frp", [H, F * WIN], f32, kind="ExternalInput").ap()
    flf = nc.dram_tensor("flf", [HH * F, HL * W], f32, kind="ExternalInput").ap()
    frf = nc.dram_tensor("frf", [HH * F, HL * W], f32, kind="ExternalInput").ap()
    p1 = nc.dram_tensor("p1", [DPC, F, H, PAD], f32, kind="ExternalInput").ap()
    p2 = nc.dram_tensor("p2", [DPC, F, H, PAD], f32, kind="ExternalInput").ap()
    olr = nc.dram_tensor("olr", [2 * F, DPC, H, W], f32, kind="ExternalOutput").ap()
    orl = nc.dram_tensor("orl", [2 * F, DPC, H, W], f32, kind="ExternalOutput").ap()

    with tile.TileContext(nc) as tc:
        with tc.tile_pool(name="stage", bufs=1) as pool:
            s_flp = pool.tile([H, F * WIN], f32, tag="s_flp")
            s_frp = pool.tile([H, F * WIN], f32, tag="s_frp")
            s_flf = pool.tile([HH * F, HL * W], f32, tag="s_flf")
            s_frf = pool.tile([HH * F, HL * W], f32, tag="s_frf")

            nc.sync.dma_start(s_flp[:], flp)
            nc.scalar.dma_start(s_frp[:], frp)
            nc.sync.dma_start(s_flf[:], flf)
            nc.scalar.dma_start(s_frf[:], frf)

            v_flp = s_flp[:].rearrange("h (f w) -> h f w", f=F)
            v_frp = s_frp[:].rearrange("h (f w) -> h f w", f=F)

            def hfw(dram_slab):
                return dram_slab.transpose([1, 0, 2])

            def afbw(dram_slab):
                return dram_slab.rearrange("f (a b) w -> a f b w", a=HH)

            import contextlib

            loop_cm = (
                tc.For_i(0, loop_reps, 1)
                if loop_reps > 1
                else contextlib.nullcontext()
            )
            with loop_cm:
              for _rep in range(reps):
                for j in range(DPC):
                    # lr-left: full-width fl (8KB runs), strip overwrites
                    nc.sync.dma_start(afbw(olr[0:F, j, :, :]), s_flf[:])
                    nc.gpsimd.dma_start(olr[0:F, j, :, 0:PAD], p1[j])
                    # lr-right: shifted window of padded fr
                    nc.scalar.dma_start(
                        hfw(olr[F : 2 * F, j, :, :]),
                        v_frp[:, :, DPC - 1 - j : DPC - 1 - j + W],
                    )
                    # rl-left: shifted window of padded fl
                    nc.sync.dma_start(
                        hfw(orl[0:F, j, :, :]), v_flp[:, :, j : j + W]
                    )
                    # rl-right: full-width fr (8KB runs), strip overwrites
                    nc.scalar.dma_start(afbw(orl[F : 2 * F, j, :, :]), s_frf[:])
                    nc.gpsimd.dma_start(orl[F : 2 * F, j, :, W - PAD : W], p2[j])

    nc.compile()
    return nc


def _get_program():
    if "nc" not in _cache:
        _cache["nc"] = _build_program()
    return _cache["nc"]


def _host_prep(fl, fr, variant=None):
    """Build the 8 per-core input maps. fl/fr: [F, H, W] f32 contiguous."""
    variant = variant or VARIANT
    if variant in ("H2", "H3"):
        # H2/H3: only the two padded windows + the column-index ramp.
        def pack(x):
            Fv, Hv, Wv = x.shape
            return np.ascontiguousarray(x.reshape(Fv * HH, HL * Wv))

        zc = np.zeros((F, HPC, PAD), dtype=np.float32)
        widr = np.tile(np.arange(PAD, dtype=np.float32), (F * HH, HL))
        in_maps = []
        for c in range(NCORES):
            h0 = HPC * c
            flc = np.ascontiguousarray(fl[:, h0 : h0 + HPC, :])
            frc = np.ascontiguousarray(fr[:, h0 : h0 + HPC, :])
            in_maps.append(
                {
                    "flp": pack(np.concatenate([flc, zc], axis=2)),
                    "frp": pack(np.concatenate([zc, frc], axis=2)),
                    "wid": widr,
                }
            )
        return in_maps

    if variant == "H":
        # H-sharding: core c gets rows [16c, 16c+16) and all 48 bins.
        # pack: [F, 16, W'] -> [F*4, 4*W'] (partition = f*4 + h_hi)
        def pack(x):
            Fv, Hv, Wv = x.shape
            return np.ascontiguousarray(x.reshape(Fv * HH, HL * Wv))

        zc = np.zeros((F, HPC, PAD), dtype=np.float32)
        widr = np.tile(np.arange(PAD, dtype=np.float32), (F * HH, HL))
        in_maps = []
        for c in range(NCORES):
            h0 = HPC * c
            flc = np.ascontiguousarray(fl[:, h0 : h0 + HPC, :])
            frc = np.ascontiguousarray(fr[:, h0 : h0 + HPC, :])
            in_maps.append(
                {
                    "fla": pack(flc),
                    "frb": pack(frc),
                    "flp": pack(np.concatenate([flc, zc], axis=2)),
                    "frp": pack(np.concatenate([zc, frc], axis=2)),
                    "flpre": pack(np.ascontiguousarray(flc[:, :, 0:PAD])),
                    "frsuf": pack(np.ascontiguousarray(frc[:, :, W - PAD : W])),
                    "wid": widr,
                }
            )
        return in_maps

    z = np.zeros((F, H, PAD), dtype=np.float32)
    fl_pad = np.concatenate([fl, z], axis=2)  # [F, H, 304]
    fr_pad = np.concatenate([z, fr], axis=2)  # [F, H, 304]

    def h_layout(x):
        # [F, H, width] -> [H, F*width]  (partition = h)
        Fv, Hv, Wv = x.shape
        return np.ascontiguousarray(np.transpose(x, (1, 0, 2)).reshape(Hv, Fv * Wv))

    def af_layout(x):
        # [F, H, width] -> [4*F, 32*width]  (partition = h_hi*F + f)
        Fv, Hv, Wv = x.shape
        hl = Hv // 4
        return np.ascontiguousarray(
            np.transpose(x.reshape(Fv, 4, hl, Wv), (1, 0, 2, 3)).reshape(
                4 * Fv, hl * Wv
            )
        )

    if variant == "B":
        to_sbuf_layout = af_layout
        extra = {"flf": af_layout(fl), "frf": af_layout(fr)}
    elif variant == "C":
        to_sbuf_layout = h_layout
        extra = {"flf": af_layout(fl), "frf": af_layout(fr)}
    elif variant == "D":
        to_sbuf_layout = h_layout
        extra = {
            "flpre": h_layout(np.ascontiguousarray(fl[:, :, 0:PAD])),
            "frsuf": h_layout(np.ascontiguousarray(fr[:, :, W - PAD : W])),
            "wid": np.tile(
                np.arange(PAD, dtype=np.float32), (H, F)
            ),  # [H, F*PAD]
        }
    else:
        to_sbuf_layout = h_layout
        extra = {
            "fl48": h_layout(np.ascontiguousarray(fl[:, :, PAD:W])),
            "fr208": h_layout(np.ascontiguousarray(fr[:, :, 0 : W - PAD])),
        }

    w48 = np.arange(PAD)  # mask index for strips
    in_maps = []
    for c in range(NCORES):
        base = DPC * c
        flp = to_sbuf_layout(fl_pad[:, :, base : base + WIN])
        frp = to_sbuf_layout(fr_pad[:, :, 43 - base : 43 - base + WIN])
        ds = base + np.arange(DPC)  # [6]
        if variant == "D":
            # thresholds per partition: [d_0..d_5, 48-d_0..48-d_5]
            # strip masks: keep fl col w  iff w >= d_j;
            #              keep fr col 208+k iff k < 48-d_j
            row = np.concatenate([ds, PAD - ds]).astype(np.float32)
            in_maps.append(
                {
                    "flp": flp,
                    "frp": frp,
                    "thr": np.ascontiguousarray(np.tile(row, (H, 1))),
                    **extra,
                }
            )
            continue
        # p1[j,f,h,w] = fl[f,h,w] if w >= d_j else 0    (w in [0,48))
        m1 = (w48[None, :] >= ds[:, None])[:, None, None, :]  # [6,1,1,48]
        p1 = np.ascontiguousarray(
            np.where(m1, fl[None, :, :, 0:PAD], np.float32(0.0)), dtype=np.float32
        )
        # p2[j,f,h,k] = fr[f,h,208+k] if 208+k < W-d_j else 0
        m2 = ((W - PAD + w48)[None, :] < (W - ds)[:, None])[:, None, None, :]
        p2 = np.ascontiguousarray(
            np.where(m2, fr[None, :, :, W - PAD : W], np.float32(0.0)),
            dtype=np.float32,
        )
        in_maps.append({"flp": flp, "frp": frp, "p1": p1, "p2": p2, **extra})
    return in_maps


def _get_exec():
    """Build (once) a persistent jitted SPMD executor for the bass program.

    Modeled on concourse.bass2jax.run_bass_via_pjrt, but cached so repeat
    calls don't re-trace/re-compile, and without output-buffer donation so
    the same callable can be invoked repeatedly (timing loops).
    """
    if "exec" in _cache:
        return _cache["exec"]

    import jax
    import concourse.mybir as mybir
    from jax.sharding import Mesh, PartitionSpec
    from jax.experimental.shard_map import shard_map
    from concourse.bass2jax import (
        _bass_exec_p,
        install_neuronx_cc_hook,
        partition_id_tensor,
    )

    nc = _get_program()
    install_neuronx_cc_hook()

    partition_name = (
        nc.partition_id_tensor.name if nc.partition_id_tensor else None
    )
    in_names, out_names, out_avals = [], [], []
    for alloc in nc.m.functions[0].allocations:
        if not isinstance(alloc, mybir.MemoryLocationSet):
            continue
        name = alloc.memorylocations[0].name
        if alloc.kind == "ExternalInput":
            if name != partition_name:
                in_names.append(name)
        elif alloc.kind == "ExternalOutput":
            out_names.append(name)
            out_avals.append(
                jax.core.ShapedArray(
                    tuple(alloc.tensor_shape), mybir.dt.np(alloc.dtype)
                )
            )
    n_params = len(in_names)
    all_names = in_names + out_names
    if partition_name is not None:
        all_names = all_names + [partition_name]

    def _body(*args):
        operands = list(args)
        if partition_name is not None:
            operands.append(partition_id_tensor())
        outs = _bass_exec_p.bind(
            *operands,
            out_avals=tuple(out_avals),
            in_names=tuple(all_names),
            out_names=tuple(out_names),
            lowering_input_output_aliases=(),
            sim_require_finite=True,
            sim_require_nnan=True,
            nc=nc,
        )
        return tuple(outs)

    devices = jax.devices()[:NCORES]
    mesh = Mesh(np.asarray(devices), ("core",))
    nin = n_params + len(out_names)
    sharded = jax.jit(
        shard_map(
            _body,
            mesh=mesh,
            in_specs=(PartitionSpec("core"),) * nin,
            out_specs=(PartitionSpec("core"),) * len(out_names),
            check_rep=False,
        ),
        keep_unused=True,
    )
    zeros = [
        np.zeros((NCORES * a.shape[0], *a.shape[1:]), a.dtype) for a in out_avals
    ]
    _cache["exec"] = (sharded, in_names, out_names, out_avals, zeros)
    return _cache["exec"]


def _run(features_left, features_right, bins, trace=False):
    fl = np.ascontiguousarray(np.asarray(features_left, dtype=np.float32)[0])
    fr = np.ascontiguousarray(np.asarray(features_right, dtype=np.float32)[0])
    in_maps = _host_prep(fl, fr)
    sharded, in_names, out_names, out_avals, zeros = _get_exec()
    concat_in = [
        np.concatenate([in_maps[c][name] for c in range(NCORES)], axis=0)
        for name in in_names
    ]
    out_arrs = sharded(*concat_in, *zeros)
    outs = {
        name: np.asarray(out_arrs[i]).reshape(NCORES, *out_avals[i].shape)
        for i, name in enumerate(out_names)
    }
    vol_lr = np.empty((B, 2 * F, D, H, W), dtype=np.float32)
    vol_rl = np.empty((B, 2 * F, D, H, W), dtype=np.float32)
    for c in range(NCORES):
        if VARIANT.startswith("H"):
            vol_lr[0, :, :, HPC * c : HPC * (c + 1)] = outs["olr"][c]
            vol_rl[0, :, :, HPC * c : HPC * (c + 1)] = outs["orl"][c]
        else:
            vol_lr[0, :, DPC * c : DPC * (c + 1)] = outs["olr"][c]
            vol_rl[0, :, DPC * c : DPC * (c + 1)] = outs["orl"][c]
    return (vol_lr, vol_rl), None


def _reference_np(features_left, features_right, bins):
    """Numpy fallback for unexpected bins (kept for robustness)."""
    fl = np.asarray(features_left, dtype=np.float32)
    fr = np.asarray(features_right, dtype=np.float32)
    bins = np.asarray(bins)
    Bv, Fv, Hv, Wv = fl.shape
    w = np.arange(Wv)
    b = bins[:, None]
    idx_m = np.clip(w[None, :] - b, 0, Wv - 1)
    idx_p = np.clip(w[None, :] + b, 0, Wv - 1)
    m_lr = (w[None, :] >= b)[None, None, :, None, :]
    m_rl = (w[None, :] < Wv - b)[None, None, :, None, :]
    g_r = np.transpose(fr[:, :, :, idx_m], (0, 1, 3, 2, 4))
    g_l = np.transpose(fl[:, :, :, idx_p], (0, 1, 3, 2, 4))
    bl = fl[:, :, None, :, :]
    br = fr[:, :, None, :, :]
    zero = np.float32(0.0)
    vol_lr = np.concatenate(
        [np.where(m_lr, bl, zero), np.where(m_lr, g_r, zero)], axis=1
    )
    vol_rl = np.concatenate(
        [np.where(m_rl, g_l, zero), np.where(m_rl, br, zero)], axis=1
    )
    return vol_lr.astype(np.float32), vol_rl.astype(np.float32)


def kernel(features_left, features_right, bins):
    fl = np.asarray(features_left)
    fr = np.asarray(features_right)
    b = np.asarray(bins)
    if (
        fl.shape != (B, F, H, W)
        or fr.shape != (B, F, H, W)
        or b.shape != (D,)
        or not np.array_equal(b, np.arange(D))
    ):
        return _reference_np(features_left, features_right, bins)
    out, _ = _run(fl, fr, b, trace=False)
    return out



# revision 17
# speedup vs baseline: 1.1166x; 1.1166x over previous
"""Trainium2 Bass kernel for ConcatVolume (stereo cost-volume concat).

Reference semantics (B=1, F=32, H=128, W=256, D=48, bins = arange(48)):
  vol_lr[0, 0:F,  d, h, w] = fl[0,:,h,w]        if w >= d      else 0
  vol_lr[0, F:2F, d, h, w] = fr[0,:,h,w-d]      if w >= d      else 0
  vol_rl[0, 0:F,  d, h, w] = fl[0,:,h,w+d]      if w <  W-d    else 0
  vol_rl[0, F:2F, d, h, w] = fr[0,:,h,w]        if w <  W-d    else 0
Returns (vol_lr, vol_rl), each [1, 2F, D, H, W] f32 (~403 MB each).

Strategy (current VARIANT, H-family): the problem is pure data movement
(~805 MB of mandatory output writes).  Shard the H axis across the 8
cores (16 rows/core, all 48 bins/core) so per-core input reads are tiny
(~1.3 MB).  SBUF layout: partition = f*4 + h_hi (128), free = (h_lo, w).
Every output byte ships in a full-width store whose DMA descriptors are
4 KB — the empirically fastest descriptor size on TRN2 (~413 GB/s/core
vs 339 GB/s for 1 KB runs).  Per bin d the four quadrant stores read
SBUF tiles prepared by the vector engine:

  lr-left  = fl with cols < d zeroed   -> masked boundary strip written
  rl-right = fr with cols >= W-d zeroed   in place (precomputed strips)
  lr-right = fr shifted by +d          -> DVE-materialized contiguous
  rl-left  = fl shifted by -d             copies of padded-window views

DVE busy time steals DMA throughput almost 1:1 (SBUF port contention,
measured), with a large per-instruction cost — so the in-loop DVE work
is collapsed into as few instructions as possible (H6: one packed
window copy + one packed strip copy per bin, via custom-stride APs) and
double-buffered 4 deep so stores never wait on it.
"""

import numpy as np

B, F, H, W, D = 1, 32, 128, 256, 48
NCORES = 8
DPC = D // NCORES  # 6 bins per core (D-sharded variants)
PAD = 48  # > max disparity (47)
WIN = W + DPC - 1  # 261: window width covering all 6 shifts
HPC = H // NCORES  # 16 rows per core (H-sharded variant)
HH, HL = 4, 4  # h = (h_hi, h_lo); partition = f*HH + h_hi
WP = W + PAD  # 304: padded width for shifted windows

_cache = {}


VARIANT = "H6"


def _build_program(reps=1, variant=None, loop_reps=1, loads_in_loop=False):
    v = variant or VARIANT
    if v == "B":
        return _build_program_b(reps)
    if v == "C":
        return _build_program_c(reps, loop_reps)
    if v == "D":
        return _build_program_d(loop_reps, loads_in_loop)
    if v == "H":
        return _build_program_h(loop_reps, loads_in_loop)
    if v == "H2":
        return _build_program_h2(loop_reps, loads_in_loop)
    if v == "H3":
        return _build_program_h3(loop_reps, loads_in_loop)
    if v == "H4":
        return _build_program_h4(loop_reps, loads_in_loop)
    if v == "H6":
        return _build_program_h6(loop_reps, loads_in_loop)
    return _build_program_a(reps, loop_reps, loads_in_loop)


def _build_program_h6(loop_reps=1, loads_in_loop=False):
    """Variant H6 = H2's all-4KB-descriptor structure with in-loop DVE
    collapsed from 4 ops/bin to 2 (probe3: DVE instructions steal DMA
    throughput with a large per-instruction cost, so fewer/bigger ops win):

    - ONE tensor_copy per bin moves BOTH shifted windows: flp|frp are
      packed in one tile and shl|shr in another, with a custom-stride
      4D AP ([p, region, hl, w], region stride = 1264-2d+...) pairing
      window(d) of flp with window(48-d) of frp;
    - ONE small tensor_copy per bin drops BOTH precomputed masked
      boundary strips (48 packed strip tiles, built once at setup) into
      the fla|frb packed tile (custom region stride 1232);
    - stores: same 4 x 4KB-desc stores per bin, round-robin on 3 queues,
      reading subviews of the packed tiles; NB=4 buffering.
    """
    import contextlib

    import concourse.bacc as bacc
    import concourse.bass as bass
    import concourse.mybir as mybir
    import concourse.tile as tile

    nc = bacc.Bacc(
        "TRN2",
        target_bir_lowering=False,
        debug=False,
        enable_asserts=False,
        num_devices=NCORES,
    )

    f32 = mybir.dt.float32
    P = F * HH  # 128 partitions
    NB = 4
    WF = HL * W  # 1024 els: one quadrant tile per partition
    WPF = HL * WP  # 1216 els: one padded tile per partition
    SF = HL * PAD  # 192 els: one strip per partition
    flp = nc.dram_tensor("flp", [P, 2 * WPF], f32, kind="ExternalInput").ap()
    wid = nc.dram_tensor("wid", [P, 2 * SF], f32, kind="ExternalInput").ap()
    pre = nc.dram_tensor("pre", [P, 2 * SF], f32, kind="ExternalInput").ap()
    olr = nc.dram_tensor("olr", [2 * F, D, HPC, W], f32, kind="ExternalOutput").ap()
    orl = nc.dram_tensor("orl", [2 * F, D, HPC, W], f32, kind="ExternalOutput").ap()

    with tile.TileContext(nc) as tc:
        with tc.tile_pool(name="stage", bufs=1) as pool:
            # packed staging: [flp | frp], [wid | wid], [flpre | frsuf]
            s_flp = pool.tile([P, 2 * WPF], f32, tag="s_flp")
            s_wid = pool.tile([P, 2 * SF], f32, tag="s_wid")
            s_pre = pool.tile([P, 2 * SF], f32, tag="s_pre")
            s_msk = pool.tile([P, 2 * SF], f32, tag="s_msk")

            def tilesN(nm, n, width):
                return [
                    pool.tile(
                        [P, width], f32, name=f"{nm}{i}", tag=f"{nm}{i}"
                    )
                    for i in range(n)
                ]

            s_ab = tilesN("s_ab", NB, 2 * WF)  # [fla | frb] packed
            s_sh = tilesN("s_sh", NB, 2 * WF)  # [shl | shr] packed
            s_st = tilesN("s_st", D, 2 * SF)  # masked strip pairs per bin

            def r4(t, b, w):
                return t[:].rearrange("p (r b w) -> p r b w", r=2, b=b)

            # one-time setup
            nc.sync.dma_start(s_flp[:], flp)
            nc.scalar.dma_start(s_wid[:], wid)
            nc.gpsimd.dma_start(s_pre[:], pre)
            # masked strip pairs: [flpre*(w>=d) | frsuf*(w<48-d)]
            for d in range(D):
                nc.vector.tensor_scalar(
                    s_msk[:, 0:SF], s_wid[:, 0:SF], float(d), None,
                    mybir.AluOpType.is_ge,
                )
                nc.vector.tensor_scalar(
                    s_msk[:, SF : 2 * SF], s_wid[:, SF : 2 * SF],
                    float(PAD - d), None, mybir.AluOpType.is_lt,
                )
                nc.vector.tensor_mul(s_st[d][:], s_pre[:], s_msk[:])
            # unshifted bodies: fla cols 48..256 / frb cols 0..208, both
            # sourced from flp cols 48..256 / frp cols 48..256
            for i in range(NB):
                vab = r4(s_ab[i], HL, W)
                vfl = r4(s_flp, HL, WP)
                nc.vector.tensor_copy(
                    vab[:, 0:1, :, PAD:W], vfl[:, 0:1, :, PAD:W]
                )
                nc.vector.tensor_copy(
                    vab[:, 1:2, :, 0 : W - PAD], vfl[:, 1:2, :, PAD:W]
                )

            def drh(slab):
                return slab.rearrange("f (a b) w -> f a b w", a=HH)

            engines = [nc.sync, nc.scalar, nc.gpsimd]

            loop_cm = (
                tc.For_i(0, loop_reps, 1)
                if loop_reps > 1
                else contextlib.nullcontext()
            )
            with loop_cm:
                if loads_in_loop:
                    nc.gpsimd.dma_start(s_flp[:], flp)
                for d in range(D):
                    i2 = d % NB
                    # ONE copy: both shifted windows.
                    # src [p, r, b, w]: r0 = flp window(d) at offset d,
                    # r1 = frp window(48-d) at WPF + 48 - d
                    src = bass.AP(
                        tensor=s_flp[:].tensor,
                        offset=d,
                        ap=[
                            (2 * WPF, P),
                            (WPF + PAD - 2 * d, 2),
                            (WP, HL),
                            (1, W),
                        ],
                    )
                    nc.vector.tensor_copy(r4(s_sh[i2], HL, W), src)
                    # ONE copy: both masked strips into fla|frb.
                    # dst r0 = fla cols 0..48, r1 = frb cols 208..256
                    dst = bass.AP(
                        tensor=s_ab[i2][:].tensor,
                        offset=0,
                        ap=[
                            (2 * WF, P),
                            (WF + W - PAD, 2),
                            (W, HL),
                            (1, PAD),
                        ],
                    )
                    nc.vector.tensor_copy(dst, r4(s_st[d], HL, PAD))
                    e = [engines[(4 * d + k) % 3] for k in range(4)]
                    # lr-left / rl-right from the packed unshifted tile
                    e[0].dma_start(
                        drh(olr[0:F, d, :, :]), s_ab[i2][:, 0:WF]
                    )
                    e[3].dma_start(
                        drh(orl[F : 2 * F, d, :, :]),
                        s_ab[i2][:, WF : 2 * WF],
                    )
                    # lr-right / rl-left from the packed shifted tile
                    e[1].dma_start(
                        drh(olr[F : 2 * F, d, :, :]),
                        s_sh[i2][:, WF : 2 * WF],
                    )
                    e[2].dma_start(
                        drh(orl[0:F, d, :, :]), s_sh[i2][:, 0:WF]
                    )

    nc.compile()
    return nc

def _build_program_a(reps=1, loop_reps=1, loads_in_loop=False):
    import concourse.bacc as bacc
    import concourse.mybir as mybir
    import concourse.tile as tile

    nc = bacc.Bacc(
        "TRN2",
        target_bir_lowering=False,
        debug=False,
        enable_asserts=False,
        num_devices=NCORES,
    )

    f32 = mybir.dt.float32
    # staging inputs come in SBUF-friendly layout [H, F*width] (host transposes)
    flp = nc.dram_tensor("flp", [H, F * WIN], f32, kind="ExternalInput").ap()
    frp = nc.dram_tensor("frp", [H, F * WIN], f32, kind="ExternalInput").ap()
    fl48 = nc.dram_tensor("fl48", [H, F * (W - PAD)], f32, kind="ExternalInput").ap()
    fr208 = nc.dram_tensor("fr208", [H, F * (W - PAD)], f32, kind="ExternalInput").ap()
    p1 = nc.dram_tensor("p1", [DPC, F, H, PAD], f32, kind="ExternalInput").ap()
    p2 = nc.dram_tensor("p2", [DPC, F, H, PAD], f32, kind="ExternalInput").ap()
    olr = nc.dram_tensor("olr", [2 * F, DPC, H, W], f32, kind="ExternalOutput").ap()
    orl = nc.dram_tensor("orl", [2 * F, DPC, H, W], f32, kind="ExternalOutput").ap()

    with tile.TileContext(nc) as tc:
        with tc.tile_pool(name="stage", bufs=1) as pool:
            # SBUF layout: partition = h (128), free = f*width + w
            s_flp = pool.tile([H, F * WIN], f32, tag="s_flp")
            s_frp = pool.tile([H, F * WIN], f32, tag="s_frp")
            s_fl48 = pool.tile([H, F * (W - PAD)], f32, tag="s_fl48")
            s_fr208 = pool.tile([H, F * (W - PAD)], f32, tag="s_fr208")

            def do_loads():
                nc.sync.dma_start(s_flp[:], flp)
                nc.scalar.dma_start(s_frp[:], frp)
                nc.sync.dma_start(s_fl48[:], fl48)
                nc.scalar.dma_start(s_fr208[:], fr208)

            if not loads_in_loop:
                do_loads()

            # SBUF views with partition (h) outermost: [h, f, w]
            v_flp = s_flp[:].rearrange("h (f w) -> h f w", f=F)
            v_frp = s_frp[:].rearrange("h (f w) -> h f w", f=F)
            v_fl48 = s_fl48[:].rearrange("h (f w) -> h f w", f=F)
            v_fr208 = s_fr208[:].rearrange("h (f w) -> h f w", f=F)

            def hfw(dram_slab):
                # DRAM slab [f, h, w] -> AP enumerated [h, f, w] to match SBUF
                return dram_slab.transpose([1, 0, 2])

            import contextlib

            loop_cm = (
                tc.For_i(0, loop_reps, 1)
                if loop_reps > 1
                else contextlib.nullcontext()
            )
            with loop_cm:
                if loads_in_loop:
                    do_loads()
                for _rep in range(reps):
                    for j in range(DPC):
                        # lr-left: cols 48.., strip covers 0..47
                        nc.sync.dma_start(hfw(olr[0:F, j, :, PAD:W]), v_fl48)
                        nc.gpsimd.dma_start(olr[0:F, j, :, 0:PAD], p1[j])
                        # lr-right: shifted window of padded fr
                        nc.scalar.dma_start(
                            hfw(olr[F : 2 * F, j, :, :]),
                            v_frp[:, :, DPC - 1 - j : DPC - 1 - j + W],
                        )
                        # rl-left: shifted window of padded fl
                        nc.sync.dma_start(
                            hfw(orl[0:F, j, :, :]), v_flp[:, :, j : j + W]
                        )
                        # rl-right: cols 0..207 from fr208, then strip p2[j]
                        nc.scalar.dma_start(
                            hfw(orl[F : 2 * F, j, :, 0 : W - PAD]), v_fr208
                        )
                        nc.gpsimd.dma_start(
                            orl[F : 2 * F, j, :, W - PAD : W], p2[j]
                        )

    nc.compile()
    return nc


def _build_program_b(reps=1):
    """Variant B: SBUF partitions = (f, h_hi) so DRAM-side store runs are
    8KB contiguous (vs 1KB in variant A). Full-width stores everywhere; the
    <=48-col boundary strips overwrite afterwards (WAW ordered by Tile)."""
    import concourse.bacc as bacc
    import concourse.mybir as mybir
    import concourse.tile as tile

    nc = bacc.Bacc(
        "TRN2",
        target_bir_lowering=False,
        debug=False,
        enable_asserts=False,
        num_devices=NCORES,
    )

    f32 = mybir.dt.float32
    HH, HL = 4, 32  # h = h_hi*HL + h_lo; partition = h_hi*F + f
    # staging inputs in [(HH*F), (HL*width)] layout (host packs)
    flp = nc.dram_tensor("flp", [HH * F, HL * WIN], f32, kind="ExternalInput").ap()
    frp = nc.dram_tensor("frp", [HH * F, HL * WIN], f32, kind="ExternalInput").ap()
    flf = nc.dram_tensor("flf", [HH * F, HL * W], f32, kind="ExternalInput").ap()
    frf = nc.dram_tensor("frf", [HH * F, HL * W], f32, kind="ExternalInput").ap()
    p1 = nc.dram_tensor("p1", [DPC, F, H, PAD], f32, kind="ExternalInput").ap()
    p2 = nc.dram_tensor("p2", [DPC, F, H, PAD], f32, kind="ExternalInput").ap()
    olr = nc.dram_tensor("olr", [2 * F, DPC, H, W], f32, kind="ExternalOutput").ap()
    orl = nc.dram_tensor("orl", [2 * F, DPC, H, W], f32, kind="ExternalOutput").ap()

    with tile.TileContext(nc) as tc:
        with tc.tile_pool(name="stage", bufs=1) as pool:
            s_flp = pool.tile([HH * F, HL * WIN], f32, tag="s_flp")
            s_frp = pool.tile([HH * F, HL * WIN], f32, tag="s_frp")
            s_flf = pool.tile([HH * F, HL * W], f32, tag="s_flf")
            s_frf = pool.tile([HH * F, HL * W], f32, tag="s_frf")

            nc.sync.dma_start(s_flp[:], flp)
            nc.scalar.dma_start(s_frp[:], frp)
            nc.sync.dma_start(s_flf[:], flf)
            nc.scalar.dma_start(s_frf[:], frf)

            # windowed views [h_hi, f, h_lo, w]
            v_flp = s_flp[:].rearrange("(a f) (b w) -> a f b w", f=F, b=HL)
            v_frp = s_frp[:].rearrange("(a f) (b w) -> a f b w", f=F, b=HL)

            for _rep in range(reps):
                for j in range(DPC):
                    # lr-left: full-width fl, strip overwrites cols 0..47
                    nc.sync.dma_start(
                        olr[0:F, j, :, :].rearrange("f (a b) w -> a f b w", a=HH),
                        s_flf[:],
                    )
                    nc.gpsimd.dma_start(olr[0:F, j, :, 0:PAD], p1[j])
                    # lr-right: shifted window of padded fr, per h_hi block
                    dst = olr[F : 2 * F, j, :, :].rearrange(
                        "f (a b) w -> a f b w", a=HH
                    )
                    s0 = DPC - 1 - j
                    for hh in range(HH):
                        nc.scalar.dma_start(
                            dst[hh], v_frp[hh, :, :, s0 : s0 + W]
                        )
                    # rl-left: shifted window of padded fl, per h_hi block
                    dst = orl[0:F, j, :, :].rearrange("f (a b) w -> a f b w", a=HH)
                    for hh in range(HH):
                        nc.sync.dma_start(dst[hh], v_flp[hh, :, :, j : j + W])
                    # rl-right: full-width fr, strip overwrites cols 208..255
                    nc.scalar.dma_start(
                        orl[F : 2 * F, j, :, :].rearrange(
                            "f (a b) w -> a f b w", a=HH
                        ),
                        s_frf[:],
                    )
                    nc.gpsimd.dma_start(orl[F : 2 * F, j, :, W - PAD : W], p2[j])

    nc.compile()
    return nc


def _build_program_c(reps=1, loop_reps=1):
    """Variant C: shifted stores as in A (partition=h, full 128-partition
    sources); the two unshifted full-width quadrants read (h_hi,f)-packed
    tiles so each is a single DMA with 8KB-contiguous DRAM runs, with the
    boundary strip overwriting afterwards."""
    import concourse.bacc as bacc
    import concourse.mybir as mybir
    import concourse.tile as tile

    nc = bacc.Bacc(
        "TRN2",
        target_bir_lowering=False,
        debug=False,
        enable_asserts=False,
        num_devices=NCORES,
    )

    f32 = mybir.dt.float32
    HH, HL = 4, 32
    flp = nc.dram_tensor("flp", [H, F * WIN], f32, kind="ExternalInput").ap()
    frp = nc.dram_tensor("frp", [H, F * WIN], f32, kind="ExternalInput").ap()
    flf = nc.dram_tensor("flf", [HH * F, HL * W], f32, kind="ExternalInput").ap()
    frf = nc.dram_tensor("frf", [HH * F, HL * W], f32, kind="ExternalInput").ap()
    p1 = nc.dram_tensor("p1", [DPC, F, H, PAD], f32, kind="ExternalInput").ap()
    p2 = nc.dram_tensor("p2", [DPC, F, H, PAD], f32, kind="ExternalInput").ap()
    olr = nc.dram_tensor("olr", [2 * F, DPC, H, W], f32, kind="ExternalOutput").ap()
    orl = nc.dram_tensor("orl", [2 * F, DPC, H, W], f32, kind="ExternalOutput").ap()

    with tile.TileContext(nc) as tc:
        with tc.tile_pool(name="stage", bufs=1) as pool:
            s_flp = pool.tile([H, F * WIN], f32, tag="s_flp")
            s_frp = pool.tile([H, F * WIN], f32, tag="s_frp")
            s_flf = pool.tile([HH * F, HL * W], f32, tag="s_flf")
            s_frf = pool.tile([HH * F, HL * W], f32, tag="s_frf")

            nc.sync.dma_start(s_flp[:], flp)
            nc.scalar.dma_start(s_frp[:], frp)
            nc.sync.dma_start(s_flf[:], flf)
            nc.scalar.dma_start(s_frf[:], frf)

            v_flp = s_flp[:].rearrange("h (f w) -> h f w", f=F)
            v_frp = s_frp[:].rearrange("h (f w) -> h f w", f=F)

            def hfw(dram_slab):
                return dram_slab.transpose([1, 0, 2])

            def afbw(dram_slab):
                return dram_slab.rearrange("f (a b) w -> a f b w", a=HH)

            import contextlib

            loop_cm = (
                tc.For_i(0, loop_reps, 1)
                if loop_reps > 1
                else contextlib.nullcontext()
            )
            with loop_cm:
              for _rep in range(reps):
                for j in range(DPC):
                    # lr-left: full-width fl (8KB runs), strip overwrites
                    nc.sync.dma_start(afbw(olr[0:F, j, :, :]), s_flf[:])
                    nc.gpsimd.dma_start(olr[0:F, j, :, 0:PAD], p1[j])
                    # lr-right: shifted window of padded fr
                    nc.scalar.dma_start(
                        hfw(olr[F : 2 * F, j, :, :]),
                        v_frp[:, :, DPC - 1 - j : DPC - 1 - j + W],
                    )
                    # rl-left: shifted window of padded fl
                    nc.sync.dma_start(
                        hfw(orl[0:F, j, :, :]), v_flp[:, :, j : j + W]
                    )
                    # rl-right: full-width fr (8KB runs), strip overwrites
                    nc.scalar.dma_start(afbw(orl[F : 2 * F, j, :, :]), s_frf[:])
                    nc.gpsimd.dma_start(orl[F : 2 * F, j, :, W - PAD : W], p2[j])

    nc.compile()
    return nc


def _get_program():
    if "nc" not in _cache:
        _cache["nc"] = _build_program()
    return _cache["nc"]


def _host_prep(fl, fr, variant=None):
    """Build the 8 per-core input maps. fl/fr: [F, H, W] f32 contiguous."""
    variant = variant or VARIANT
    if variant == "H6":
        # H6: packed [flp|frp], [wid|wid], [flpre|frsuf]
        def pack_h(x):
            Fv, Hv, Wv = x.shape
            return np.ascontiguousarray(x.reshape(Fv * HH, HL * Wv))

        zc = np.zeros((F, HPC, PAD), dtype=np.float32)
        widr = np.tile(np.arange(PAD, dtype=np.float32), (F * HH, 2 * HL))
        in_maps = []
        for c in range(NCORES):
            h0 = HPC * c
            flc = np.ascontiguousarray(fl[:, h0 : h0 + HPC, :])
            frc = np.ascontiguousarray(fr[:, h0 : h0 + HPC, :])
            in_maps.append(
                {
                    "flp": np.concatenate(
                        [
                            pack_h(np.concatenate([flc, zc], axis=2)),
                            pack_h(np.concatenate([zc, frc], axis=2)),
                        ],
                        axis=1,
                    ),
                    "wid": widr,
                    "pre": np.concatenate(
                        [
                            pack_h(np.ascontiguousarray(flc[:, :, 0:PAD])),
                            pack_h(
                                np.ascontiguousarray(frc[:, :, W - PAD : W])
                            ),
                        ],
                        axis=1,
                    ),
                }
            )
        return in_maps

    if variant == "H4":
        def pack_h(x):
            Fv, Hv, Wv = x.shape
            return np.ascontiguousarray(x.reshape(Fv * HH, HL * Wv))

        def pack_w(x):
            # [F, HPC, W'] -> w-major [F, W'*HPC] (free = (w, h))
            Fv, Hv, Wv = x.shape
            return np.ascontiguousarray(
                np.transpose(x, (0, 2, 1)).reshape(Fv, Wv * Hv)
            )

        zc = np.zeros((F, HPC, PAD), dtype=np.float32)
        widr = np.tile(np.arange(PAD, dtype=np.float32), (F * HH, HL))
        in_maps = []
        for c in range(NCORES):
            h0 = HPC * c
            flc = np.ascontiguousarray(fl[:, h0 : h0 + HPC, :])
            frc = np.ascontiguousarray(fr[:, h0 : h0 + HPC, :])
            in_maps.append(
                {
                    "fla": pack_h(flc),
                    "frb": pack_h(frc),
                    "flw": pack_w(np.concatenate([flc, zc], axis=2)),
                    "frw": pack_w(np.concatenate([zc, frc], axis=2)),
                    "flpre": pack_h(np.ascontiguousarray(flc[:, :, 0:PAD])),
                    "frsuf": pack_h(
                        np.ascontiguousarray(frc[:, :, W - PAD : W])
                    ),
                    "wid": widr,
                }
            )
        return in_maps

    if variant in ("H2", "H3"):
        # H2/H3: only the two padded windows + the column-index ramp.
        def pack(x):
            Fv, Hv, Wv = x.shape
            return np.ascontiguousarray(x.reshape(Fv * HH, HL * Wv))

        zc = np.zeros((F, HPC, PAD), dtype=np.float32)
        widr = np.tile(np.arange(PAD, dtype=np.float32), (F * HH, HL))
        in_maps = []
        for c in range(NCORES):
            h0 = HPC * c
            flc = np.ascontiguousarray(fl[:, h0 : h0 + HPC, :])
            frc = np.ascontiguousarray(fr[:, h0 : h0 + HPC, :])
            in_maps.append(
                {
                    "flp": pack(np.concatenate([flc, zc], axis=2)),
                    "frp": pack(np.concatenate([zc, frc], axis=2)),
                    "wid": widr,
                }
            )
        return in_maps

    if variant == "H":
        # H-sharding: core c gets rows [16c, 16c+16) and all 48 bins.
        # pack: [F, 16, W'] -> [F*4, 4*W'] (partition = f*4 + h_hi)
        def pack(x):
            Fv, Hv, Wv = x.shape
            return np.ascontiguousarray(x.reshape(Fv * HH, HL * Wv))

        zc = np.zeros((F, HPC, PAD), dtype=np.float32)
        widr = np.tile(np.arange(PAD, dtype=np.float32), (F * HH, HL))
        in_maps = []
        for c in range(NCORES):
            h0 = HPC * c
            flc = np.ascontiguousarray(fl[:, h0 : h0 + HPC, :])
            frc = np.ascontiguousarray(fr[:, h0 : h0 + HPC, :])
            in_maps.append(
                {
                    "fla": pack(flc),
                    "frb": pack(frc),
                    "flp": pack(np.concatenate([flc, zc], axis=2)),
                    "frp": pack(np.concatenate([zc, frc], axis=2)),
                    "flpre": pack(np.ascontiguousarray(flc[:, :, 0:PAD])),
                    "frsuf": pack(np.ascontiguousarray(frc[:, :, W - PAD : W])),
                    "wid": widr,
                }
            )
        return in_maps

    z = np.zeros((F, H, PAD), dtype=np.float32)
    fl_pad = np.concatenate([fl, z], axis=2)  # [F, H, 304]
    fr_pad = np.concatenate([z, fr], axis=2)  # [F, H, 304]

    def h_layout(x):
        # [F, H, width] -> [H, F*width]  (partition = h)
        Fv, Hv, Wv = x.shape
        return np.ascontiguousarray(np.transpose(x, (1, 0, 2)).reshape(Hv, Fv * Wv))

    def af_layout(x):
        # [F, H, width] -> [4*F, 32*width]  (partition = h_hi*F + f)
        Fv, Hv, Wv = x.shape
        hl = Hv // 4
        return np.ascontiguousarray(
            np.transpose(x.reshape(Fv, 4, hl, Wv), (1, 0, 2, 3)).reshape(
                4 * Fv, hl * Wv
            )
        )

    if variant == "B":
        to_sbuf_layout = af_layout
        extra = {"flf": af_layout(fl), "frf": af_layout(fr)}
    elif variant == "C":
        to_sbuf_layout = h_layout
        extra = {"flf": af_layout(fl), "frf": af_layout(fr)}
    elif variant == "D":
        to_sbuf_layout = h_layout
        extra = {
            "flpre": h_layout(np.ascontiguousarray(fl[:, :, 0:PAD])),
            "frsuf": h_layout(np.ascontiguousarray(fr[:, :, W - PAD : W])),
            "wid": np.tile(
                np.arange(PAD, dtype=np.float32), (H, F)
            ),  # [H, F*PAD]
        }
    else:
        to_sbuf_layout = h_layout
        extra = {
            "fl48": h_layout(np.ascontiguousarray(fl[:, :, PAD:W])),
            "fr208": h_layout(np.ascontiguousarray(fr[:, :, 0 : W - PAD])),
        }

    w48 = np.arange(PAD)  # mask index for strips
    in_maps = []
    for c in range(NCORES):
        base = DPC * c
        flp = to_sbuf_layout(fl_pad[:, :, base : base + WIN])
        frp = to_sbuf_layout(fr_pad[:, :, 43 - base : 43 - base + WIN])
        ds = base + np.arange(DPC)  # [6]
        if variant == "D":
            # thresholds per partition: [d_0..d_5, 48-d_0..48-d_5]
            # strip masks: keep fl col w  iff w >= d_j;
            #              keep fr col 208+k iff k < 48-d_j
            row = np.concatenate([ds, PAD - ds]).astype(np.float32)
            in_maps.append(
                {
                    "flp": flp,
                    "frp": frp,
                    "thr": np.ascontiguousarray(np.tile(row, (H, 1))),
                    **extra,
                }
            )
            continue
        # p1[j,f,h,w] = fl[f,h,w] if w >= d_j else 0    (w in [0,48))
        m1 = (w48[None, :] >= ds[:, None])[:, None, None, :]  # [6,1,1,48]
        p1 = np.ascontiguousarray(
            np.where(m1, fl[None, :, :, 0:PAD], np.float32(0.0)), dtype=np.float32
        )
        # p2[j,f,h,k] = fr[f,h,208+k] if 208+k < W-d_j else 0
        m2 = ((W - PAD + w48)[None, :] < (W - ds)[:, None])[:, None, None, :]
        p2 = np.ascontiguousarray(
            np.where(m2, fr[None, :, :, W - PAD : W], np.float32(0.0)),
            dtype=np.float32,
        )
        in_maps.append({"flp": flp, "frp": frp, "p1": p1, "p2": p2, **extra})
    return in_maps


def _get_exec():
    """Build (once) a persistent jitted SPMD executor for the bass program.

    Modeled on concourse.bass2jax.run_bass_via_pjrt, but cached so repeat
    calls don't re-trace/re-compile, and without output-buffer donation so
    the same callable can be invoked repeatedly (timing loops).
    """
    if "exec" in _cache:
        return _cache["exec"]

    import jax
    import concourse.mybir as mybir
    from jax.sharding import Mesh, PartitionSpec
    from jax.experimental.shard_map import shard_map
    from concourse.bass2jax import (
        _bass_exec_p,
        install_neuronx_cc_hook,
        partition_id_tensor,
    )

    nc = _get_program()
    install_neuronx_cc_hook()

    partition_name = (
        nc.partition_id_tensor.name if nc.partition_id_tensor else None
    )
    in_names, out_names, out_avals = [], [], []
    for alloc in nc.m.functions[0].allocations:
        if not isinstance(alloc, mybir.MemoryLocationSet):
            continue
        name = alloc.memorylocations[0].name
        if alloc.kind == "ExternalInput":
            if name != partition_name:
                in_names.append(name)
        elif alloc.kind == "ExternalOutput":
            out_names.append(name)
            out_avals.append(
                jax.core.ShapedArray(
                    tuple(alloc.tensor_shape), mybir.dt.np(alloc.dtype)
                )
            )
    n_params = len(in_names)
    all_names = in_names + out_names
    if partition_name is not None:
        all_names = all_names + [partition_name]

    def _body(*args):
        operands = list(args)
        if partition_name is not None:
            operands.append(partition_id_tensor())
        outs = _bass_exec_p.bind(
            *operands,
            out_avals=tuple(out_avals),
            in_names=tuple(all_names),
            out_names=tuple(out_names),
            lowering_input_output_aliases=(),
            sim_require_finite=True,
            sim_require_nnan=True,
            nc=nc,
        )
        return tuple(outs)

    devices = jax.devices()[:NCORES]
    mesh = Mesh(np.asarray(devices), ("core",))
    nin = n_params + len(out_names)
    sharded = jax.jit(
        shard_map(
            _body,
            mesh=mesh,
            in_specs=(PartitionSpec("core"),) * nin,
            out_specs=(PartitionSpec("core"),) * len(out_names),
            check_rep=False,
        ),
        keep_unused=True,
    )
    zeros = [
        np.zeros((NCORES * a.shape[0], *a.shape[1:]), a.dtype) for a in out_avals
    ]
    _cache["exec"] = (sharded, in_names, out_names, out_avals, zeros)
    return _cache["exec"]


def _run(features_left, features_right, bins, trace=False):
    fl = np.ascontiguousarray(np.asarray(features_left, dtype=np.float32)[0])
    fr = np.ascontiguousarray(np.asarray(features_right, dtype=np.float32)[0])
    in_maps = _host_prep(fl, fr)
    sharded, in_names, out_names, out_avals, zeros = _get_exec()
    concat_in = [
        np.concatenate([in_maps[c][name] for c in range(NCORES)], axis=0)
        for name in in_names
    ]
    out_arrs = sharded(*concat_in, *zeros)
    outs = {
        name: np.asarray(out_arrs[i]).reshape(NCORES, *out_avals[i].shape)
        for i, name in enumerate(out_names)
    }
    vol_lr = np.empty((B, 2 * F, D, H, W), dtype=np.float32)
    vol_rl = np.empty((B, 2 * F, D, H, W), dtype=np.float32)
    for c in range(NCORES):
        if VARIANT.startswith("H"):
            vol_lr[0, :, :, HPC * c : HPC * (c + 1)] = outs["olr"][c]
            vol_rl[0, :, :, HPC * c : HPC * (c + 1)] = outs["orl"][c]
        else:
            vol_lr[0, :, DPC * c : DPC * (c + 1)] = outs["olr"][c]
            vol_rl[0, :, DPC * c : DPC * (c + 1)] = outs["orl"][c]
    return (vol_lr, vol_rl), None


def _reference_np(features_left, features_right, bins):
    """Numpy fallback for unexpected bins (kept for robustness)."""
    fl = np.asarray(features_left, dtype=np.float32)
    fr = np.asarray(features_right, dtype=np.float32)
    bins = np.asarray(bins)
    Bv, Fv, Hv, Wv = fl.shape
    w = np.arange(Wv)
    b = bins[:, None]
    idx_m = np.clip(w[None, :] - b, 0, Wv - 1)
    idx_p = np.clip(w[None, :] + b, 0, Wv - 1)
    m_lr = (w[None, :] >= b)[None, None, :, None, :]
    m_rl = (w[None, :] < Wv - b)[None, None, :, None, :]
    g_r = np.transpose(fr[:, :, :, idx_m], (0, 1, 3, 2, 4))
    g_l = np.transpose(fl[:, :, :, idx_p], (0, 1, 3, 2, 4))
    bl = fl[:, :, None, :, :]
    br = fr[:, :, None, :, :]
    zero = np.float32(0.0)
    vol_lr = np.concatenate(
        [np.where(m_lr, bl, zero), np.where(m_lr, g_r, zero)], axis=1
    )
    vol_rl = np.concatenate(
        [np.where(m_rl, g_l, zero), np.where(m_rl, br, zero)], axis=1
    )
    return vol_lr.astype(np.float32), vol_rl.astype(np.float32)


def kernel(features_left, features_right, bins):
    fl = np.asarray(features_left)
    fr = np.asarray(features_right)
    b = np.asarray(bins)
    if (
        fl.shape != (B, F, H, W)
        or fr.shape != (B, F, H, W)
        or b.shape != (D,)
        or not np.array_equal(b, np.arange(D))
    ):
        return _reference_np(features_left, features_right, bins)
    out, _ = _run(fl, fr, b, trace=False)
    return out



# revision 18
# speedup vs baseline: 1.3595x; 1.2176x over previous
"""Trainium2 Bass kernel for ConcatVolume (stereo cost-volume concat).

Reference semantics (B=1, F=32, H=128, W=256, D=48, bins = arange(48)):
  vol_lr[0, 0:F,  d, h, w] = fl[0,:,h,w]        if w >= d      else 0
  vol_lr[0, F:2F, d, h, w] = fr[0,:,h,w-d]      if w >= d      else 0
  vol_rl[0, 0:F,  d, h, w] = fl[0,:,h,w+d]      if w <  W-d    else 0
  vol_rl[0, F:2F, d, h, w] = fr[0,:,h,w]        if w <  W-d    else 0
Returns (vol_lr, vol_rl), each [1, 2F, D, H, W] f32 (~403 MB each).

Strategy (current VARIANT, H-family): the problem is pure data movement
(~805 MB of mandatory output writes).  Shard the H axis across the 8
cores (16 rows/core, all 48 bins/core) so per-core input reads are tiny
(~1.3 MB).  SBUF layout: partition = f*4 + h_hi (128), free = (h_lo, w).
Every output byte ships in a full-width store whose DMA descriptors are
4 KB — the empirically fastest descriptor size on TRN2 (~413 GB/s/core
vs 339 GB/s for 1 KB runs).  Per bin d the four quadrant stores read
SBUF tiles prepared by the vector engine:

  lr-left  = fl with cols < d zeroed   -> masked boundary strip written
  rl-right = fr with cols >= W-d zeroed   in place (precomputed strips)
  lr-right = fr shifted by +d          -> DVE-materialized contiguous
  rl-left  = fl shifted by -d             copies of padded-window views

DVE busy time steals DMA throughput almost 1:1 (SBUF port contention,
measured), with a large per-instruction cost — so the in-loop DVE work
is collapsed into as few instructions as possible (H6: one packed
window copy + one packed strip copy per bin, via custom-stride APs) and
double-buffered 4 deep so stores never wait on it.
"""

import numpy as np

B, F, H, W, D = 1, 32, 128, 256, 48
NCORES = 8
DPC = D // NCORES  # 6 bins per core (D-sharded variants)
PAD = 48  # > max disparity (47)
WIN = W + DPC - 1  # 261: window width covering all 6 shifts
HPC = H // NCORES  # 16 rows per core (H-sharded variant)
HH, HL = 4, 4  # h = (h_hi, h_lo); partition = f*HH + h_hi
WP = W + PAD  # 304: padded width for shifted windows

_cache = {}


VARIANT = "H6"


def _build_program(reps=1, variant=None, loop_reps=1, loads_in_loop=False):
    v = variant or VARIANT
    if v == "B":
        return _build_program_b(reps)
    if v == "C":
        return _build_program_c(reps, loop_reps)
    if v == "H6":
        return _build_program_h6(loop_reps, loads_in_loop)
    return _build_program_a(reps, loop_reps, loads_in_loop)


def _build_program_h6(loop_reps=1, loads_in_loop=False):
    """Variant H6 = H2's all-4KB-descriptor structure with in-loop DVE
    collapsed from 4 ops/bin to 2 (probe3: DVE instructions steal DMA
    throughput with a large per-instruction cost, so fewer/bigger ops win):

    - ONE tensor_copy per bin moves BOTH shifted windows: flp|frp are
      packed in one tile and shl|shr in another, with a custom-stride
      4D AP ([p, region, hl, w], region stride = 1264-2d+...) pairing
      window(d) of flp with window(48-d) of frp;
    - ONE small tensor_copy per bin drops BOTH precomputed masked
      boundary strips (48 packed strip tiles, built once at setup) into
      the fla|frb packed tile (custom region stride 1232);
    - stores: same 4 x 4KB-desc stores per bin, round-robin on 3 queues,
      reading subviews of the packed tiles; NB=4 buffering.
    """
    import contextlib

    import concourse.bacc as bacc
    import concourse.bass as bass
    import concourse.mybir as mybir
    import concourse.tile as tile

    nc = bacc.Bacc(
        "TRN2",
        target_bir_lowering=False,
        debug=False,
        enable_asserts=False,
        num_devices=NCORES,
    )

    f32 = mybir.dt.float32
    P = F * HH  # 128 partitions
    NB = 4
    WF = HL * W  # 1024 els: one quadrant tile per partition
    WPF = HL * WP  # 1216 els: one padded tile per partition
    SF = HL * PAD  # 192 els: one strip per partition
    flp = nc.dram_tensor("flp", [P, 2 * WPF], f32, kind="ExternalInput").ap()
    wid = nc.dram_tensor("wid", [P, 2 * SF], f32, kind="ExternalInput").ap()
    pre = nc.dram_tensor("pre", [P, 2 * SF], f32, kind="ExternalInput").ap()
    olr = nc.dram_tensor("olr", [2 * F, D, HPC, W], f32, kind="ExternalOutput").ap()
    orl = nc.dram_tensor("orl", [2 * F, D, HPC, W], f32, kind="ExternalOutput").ap()

    with tile.TileContext(nc) as tc:
        with tc.tile_pool(name="stage", bufs=1) as pool:
            # packed staging: [flp | frp], [wid | wid], [flpre | frsuf]
            s_flp = pool.tile([P, 2 * WPF], f32, tag="s_flp")
            s_wid = pool.tile([P, 2 * SF], f32, tag="s_wid")
            s_pre = pool.tile([P, 2 * SF], f32, tag="s_pre")
            s_msk = pool.tile([P, 2 * SF], f32, tag="s_msk")

            def tilesN(nm, n, width):
                return [
                    pool.tile(
                        [P, width], f32, name=f"{nm}{i}", tag=f"{nm}{i}"
                    )
                    for i in range(n)
                ]

            s_ab = tilesN("s_ab", NB, 2 * WF)  # [fla | frb] packed
            s_sh = tilesN("s_sh", NB, 2 * WF)  # [shl | shr] packed
            s_st = tilesN("s_st", D, 2 * SF)  # masked strip pairs per bin

            def r4(t, b, w):
                return t[:].rearrange("p (r b w) -> p r b w", r=2, b=b)

            # one-time setup
            nc.sync.dma_start(s_flp[:], flp)
            nc.scalar.dma_start(s_wid[:], wid)
            nc.gpsimd.dma_start(s_pre[:], pre)
            # masked strip pairs: [flpre*(w>=d) | frsuf*(w<48-d)]
            for d in range(D):
                nc.vector.tensor_scalar(
                    s_msk[:, 0:SF], s_wid[:, 0:SF], float(d), None,
                    mybir.AluOpType.is_ge,
                )
                nc.vector.tensor_scalar(
                    s_msk[:, SF : 2 * SF], s_wid[:, SF : 2 * SF],
                    float(PAD - d), None, mybir.AluOpType.is_lt,
                )
                nc.vector.tensor_mul(s_st[d][:], s_pre[:], s_msk[:])
            # unshifted bodies: fla cols 48..256 / frb cols 0..208, both
            # sourced from flp cols 48..256 / frp cols 48..256
            for i in range(NB):
                vab = r4(s_ab[i], HL, W)
                vfl = r4(s_flp, HL, WP)
                nc.vector.tensor_copy(
                    vab[:, 0:1, :, PAD:W], vfl[:, 0:1, :, PAD:W]
                )
                nc.vector.tensor_copy(
                    vab[:, 1:2, :, 0 : W - PAD], vfl[:, 1:2, :, PAD:W]
                )

            def drh(slab):
                return slab.rearrange("f (a b) w -> f a b w", a=HH)

            engines = [nc.sync, nc.scalar, nc.gpsimd]

            loop_cm = (
                tc.For_i(0, loop_reps, 1)
                if loop_reps > 1
                else contextlib.nullcontext()
            )
            with loop_cm:
                if loads_in_loop:
                    nc.gpsimd.dma_start(s_flp[:], flp)
                for d in range(D):
                    i2 = d % NB
                    # ONE copy: both shifted windows.
                    # src [p, r, b, w]: r0 = flp window(d) at offset d,
                    # r1 = frp window(48-d) at WPF + 48 - d
                    src = bass.AP(
                        tensor=s_flp[:].tensor,
                        offset=d,
                        ap=[
                            (2 * WPF, P),
                            (WPF + PAD - 2 * d, 2),
                            (WP, HL),
                            (1, W),
                        ],
                    )
                    nc.vector.tensor_copy(r4(s_sh[i2], HL, W), src)
                    # ONE copy: both masked strips into fla|frb.
                    # dst r0 = fla cols 0..48, r1 = frb cols 208..256
                    dst = bass.AP(
                        tensor=s_ab[i2][:].tensor,
                        offset=0,
                        ap=[
                            (2 * WF, P),
                            (WF + W - PAD, 2),
                            (W, HL),
                            (1, PAD),
                        ],
                    )
                    nc.vector.tensor_copy(dst, r4(s_st[d], HL, PAD))
                    e = [engines[(4 * d + k) % 3] for k in range(4)]
                    # lr-left / rl-right from the packed unshifted tile
                    e[0].dma_start(
                        drh(olr[0:F, d, :, :]), s_ab[i2][:, 0:WF]
                    )
                    e[3].dma_start(
                        drh(orl[F : 2 * F, d, :, :]),
                        s_ab[i2][:, WF : 2 * WF],
                    )
                    # lr-right / rl-left from the packed shifted tile
                    e[1].dma_start(
                        drh(olr[F : 2 * F, d, :, :]),
                        s_sh[i2][:, WF : 2 * WF],
                    )
                    e[2].dma_start(
                        drh(orl[0:F, d, :, :]), s_sh[i2][:, 0:WF]
                    )

    nc.compile()
    return nc

def _build_program_a(reps=1, loop_reps=1, loads_in_loop=False):
    import concourse.bacc as bacc
    import concourse.mybir as mybir
    import concourse.tile as tile

    nc = bacc.Bacc(
        "TRN2",
        target_bir_lowering=False,
        debug=False,
        enable_asserts=False,
        num_devices=NCORES,
    )

    f32 = mybir.dt.float32
    # staging inputs come in SBUF-friendly layout [H, F*width] (host transposes)
    flp = nc.dram_tensor("flp", [H, F * WIN], f32, kind="ExternalInput").ap()
    frp = nc.dram_tensor("frp", [H, F * WIN], f32, kind="ExternalInput").ap()
    fl48 = nc.dram_tensor("fl48", [H, F * (W - PAD)], f32, kind="ExternalInput").ap()
    fr208 = nc.dram_tensor("fr208", [H, F * (W - PAD)], f32, kind="ExternalInput").ap()
    p1 = nc.dram_tensor("p1", [DPC, F, H, PAD], f32, kind="ExternalInput").ap()
    p2 = nc.dram_tensor("p2", [DPC, F, H, PAD], f32, kind="ExternalInput").ap()
    olr = nc.dram_tensor("olr", [2 * F, DPC, H, W], f32, kind="ExternalOutput").ap()
    orl = nc.dram_tensor("orl", [2 * F, DPC, H, W], f32, kind="ExternalOutput").ap()

    with tile.TileContext(nc) as tc:
        with tc.tile_pool(name="stage", bufs=1) as pool:
            # SBUF layout: partition = h (128), free = f*width + w
            s_flp = pool.tile([H, F * WIN], f32, tag="s_flp")
            s_frp = pool.tile([H, F * WIN], f32, tag="s_frp")
            s_fl48 = pool.tile([H, F * (W - PAD)], f32, tag="s_fl48")
            s_fr208 = pool.tile([H, F * (W - PAD)], f32, tag="s_fr208")

            def do_loads():
                nc.sync.dma_start(s_flp[:], flp)
                nc.scalar.dma_start(s_frp[:], frp)
                nc.sync.dma_start(s_fl48[:], fl48)
                nc.scalar.dma_start(s_fr208[:], fr208)

            if not loads_in_loop:
                do_loads()

            # SBUF views with partition (h) outermost: [h, f, w]
            v_flp = s_flp[:].rearrange("h (f w) -> h f w", f=F)
            v_frp = s_frp[:].rearrange("h (f w) -> h f w", f=F)
            v_fl48 = s_fl48[:].rearrange("h (f w) -> h f w", f=F)
            v_fr208 = s_fr208[:].rearrange("h (f w) -> h f w", f=F)

            def hfw(dram_slab):
                # DRAM slab [f, h, w] -> AP enumerated [h, f, w] to match SBUF
                return dram_slab.transpose([1, 0, 2])

            import contextlib

            loop_cm = (
                tc.For_i(0, loop_reps, 1)
                if loop_reps > 1
                else contextlib.nullcontext()
            )
            with loop_cm:
                if loads_in_loop:
                    do_loads()
                for _rep in range(reps):
                    for j in range(DPC):
                        # lr-left: cols 48.., strip covers 0..47
                        nc.sync.dma_start(hfw(olr[0:F, j, :, PAD:W]), v_fl48)
                        nc.gpsimd.dma_start(olr[0:F, j, :, 0:PAD], p1[j])
                        # lr-right: shifted window of padded fr
                        nc.scalar.dma_start(
                            hfw(olr[F : 2 * F, j, :, :]),
                            v_frp[:, :, DPC - 1 - j : DPC - 1 - j + W],
                        )
                        # rl-left: shifted window of padded fl
                        nc.sync.dma_start(
                            hfw(orl[0:F, j, :, :]), v_flp[:, :, j : j + W]
                        )
                        # rl-right: cols 0..207 from fr208, then strip p2[j]
                        nc.scalar.dma_start(
                            hfw(orl[F : 2 * F, j, :, 0 : W - PAD]), v_fr208
                        )
                        nc.gpsimd.dma_start(
                            orl[F : 2 * F, j, :, W - PAD : W], p2[j]
                        )

    nc.compile()
    return nc


def _build_program_b(reps=1):
    """Variant B: SBUF partitions = (f, h_hi) so DRAM-side store runs are
    8KB contiguous (vs 1KB in variant A). Full-width stores everywhere; the
    <=48-col boundary strips overwrite afterwards (WAW ordered by Tile)."""
    import concourse.bacc as bacc
    import concourse.mybir as mybir
    import concourse.tile as tile

    nc = bacc.Bacc(
        "TRN2",
        target_bir_lowering=False,
        debug=False,
        enable_asserts=False,
        num_devices=NCORES,
    )

    f32 = mybir.dt.float32
    HH, HL = 4, 32  # h = h_hi*HL + h_lo; partition = h_hi*F + f
    # staging inputs in [(HH*F), (HL*width)] layout (host packs)
    flp = nc.dram_tensor("flp", [HH * F, HL * WIN], f32, kind="ExternalInput").ap()
    frp = nc.dram_tensor("frp", [HH * F, HL * WIN], f32, kind="ExternalInput").ap()
    flf = nc.dram_tensor("flf", [HH * F, HL * W], f32, kind="ExternalInput").ap()
    frf = nc.dram_tensor("frf", [HH * F, HL * W], f32, kind="ExternalInput").ap()
    p1 = nc.dram_tensor("p1", [DPC, F, H, PAD], f32, kind="ExternalInput").ap()
    p2 = nc.dram_tensor("p2", [DPC, F, H, PAD], f32, kind="ExternalInput").ap()
    olr = nc.dram_tensor("olr", [2 * F, DPC, H, W], f32, kind="ExternalOutput").ap()
    orl = nc.dram_tensor("orl", [2 * F, DPC, H, W], f32, kind="ExternalOutput").ap()

    with tile.TileContext(nc) as tc:
        with tc.tile_pool(name="stage", bufs=1) as pool:
            s_flp = pool.tile([HH * F, HL * WIN], f32, tag="s_flp")
            s_frp = pool.tile([HH * F, HL * WIN], f32, tag="s_frp")
            s_flf = pool.tile([HH * F, HL * W], f32, tag="s_flf")
            s_frf = pool.tile([HH * F, HL * W], f32, tag="s_frf")

            nc.sync.dma_start(s_flp[:], flp)
            nc.scalar.dma_start(s_frp[:], frp)
            nc.sync.dma_start(s_flf[:], flf)
            nc.scalar.dma_start(s_frf[:], frf)

            # windowed views [h_hi, f, h_lo, w]
            v_flp = s_flp[:].rearrange("(a f) (b w) -> a f b w", f=F, b=HL)
            v_frp = s_frp[:].rearrange("(a f) (b w) -> a f b w", f=F, b=HL)

            for _rep in range(reps):
                for j in range(DPC):
                    # lr-left: full-width fl, strip overwrites cols 0..47
                    nc.sync.dma_start(
                        olr[0:F, j, :, :].rearrange("f (a b) w -> a f b w", a=HH),
                        s_flf[:],
                    )
                    nc.gpsimd.dma_start(olr[0:F, j, :, 0:PAD], p1[j])
                    # lr-right: shifted window of padded fr, per h_hi block
                    dst = olr[F : 2 * F, j, :, :].rearrange(
                        "f (a b) w -> a f b w", a=HH
                    )
                    s0 = DPC - 1 - j
                    for hh in range(HH):
                        nc.scalar.dma_start(
                            dst[hh], v_frp[hh, :, :, s0 : s0 + W]
                        )
                    # rl-left: shifted window of padded fl, per h_hi block
                    dst = orl[0:F, j, :, :].rearrange("f (a b) w -> a f b w", a=HH)
                    for hh in range(HH):
                        nc.sync.dma_start(dst[hh], v_flp[hh, :, :, j : j + W])
                    # rl-right: full-width fr, strip overwrites cols 208..255
                    nc.scalar.dma_start(
                        orl[F : 2 * F, j, :, :].rearrange(
                            "f (a b) w -> a f b w", a=HH
                        ),
                        s_frf[:],
                    )
                    nc.gpsimd.dma_start(orl[F : 2 * F, j, :, W - PAD : W], p2[j])

    nc.compile()
    return nc


def _build_program_c(reps=1, loop_reps=1):
    """Variant C: shifted stores as in A (partition=h, full 128-partition
    sources); the two unshifted full-width quadrants read (h_hi,f)-packed
    tiles so each is a single DMA with 8KB-contiguous DRAM runs, with the
    boundary strip overwriting afterwards."""
    import concourse.bacc as bacc
    import concourse.mybir as mybir
    import concourse.tile as tile

    nc = bacc.Bacc(
        "TRN2",
        target_bir_lowering=False,
        debug=False,
        enable_asserts=False,
        num_devices=NCORES,
    )

    f32 = mybir.dt.float32
    HH, HL = 4, 32
    flp = nc.dram_tensor("flp", [H, F * WIN], f32, kind="ExternalInput").ap()
    frp = nc.dram_tensor("frp", [H, F * WIN], f32, kind="ExternalInput").ap()
    flf = nc.dram_tensor("flf", [HH * F, HL * W], f32, kind="ExternalInput").ap()
    frf = nc.dram_tensor("frf", [HH * F, HL * W], f32, kind="ExternalInput").ap()
    p1 = nc.dram_tensor("p1", [DPC, F, H, PAD], f32, kind="ExternalInput").ap()
    p2 = nc.dram_tensor("p2", [DPC, F, H, PAD], f32, kind="ExternalInput").ap()
    olr = nc.dram_tensor("olr", [2 * F, DPC, H, W], f32, kind="ExternalOutput").ap()
    orl = nc.dram_tensor("orl", [2 * F, DPC, H, W], f32, kind="ExternalOutput").ap()

    with tile.TileContext(nc) as tc:
        with tc.tile_pool(name="stage", bufs=1) as pool:
            s_flp = pool.tile([H, F * WIN], f32, tag="s_flp")
            s_frp = pool.tile([H, F * WIN], f32, tag="s_frp")
            s_flf = pool.tile([HH * F, HL * W], f32, tag="s_flf")
            s_frf = pool.tile([HH * F, HL * W], f32, tag="s_frf")

            nc.sync.dma_start(s_flp[:], flp)
            nc.scalar.dma_start(s_frp[:], frp)
            nc.sync.dma_start(s_flf[:], flf)
            nc.scalar.dma_start(s_frf[:], frf)

            v_flp = s_flp[:].rearrange("h (f w) -> h f w", f=F)
            v_frp = s_frp[:].rearrange("h (f w) -> h f w", f=F)

            def hfw(dram_slab):
                return dram_slab.transpose([1, 0, 2])

            def afbw(dram_slab):
                return dram_slab.rearrange("f (a b) w -> a f b w", a=HH)

            import contextlib

            loop_cm = (
                tc.For_i(0, loop_reps, 1)
                if loop_reps > 1
                else contextlib.nullcontext()
            )
            with loop_cm:
              for _rep in range(reps):
                for j in range(DPC):
                    # lr-left: full-width fl (8KB runs), strip overwrites
                    nc.sync.dma_start(afbw(olr[0:F, j, :, :]), s_flf[:])
                    nc.gpsimd.dma_start(olr[0:F, j, :, 0:PAD], p1[j])
                    # lr-right: shifted window of padded fr
                    nc.scalar.dma_start(
                        hfw(olr[F : 2 * F, j, :, :]),
                        v_frp[:, :, DPC - 1 - j : DPC - 1 - j + W],
                    )
                    # rl-left: shifted window of padded fl
                    nc.sync.dma_start(
                        hfw(orl[0:F, j, :, :]), v_flp[:, :, j : j + W]
                    )
                    # rl-right: full-width fr (8KB runs), strip overwrites
                    nc.scalar.dma_start(afbw(orl[F : 2 * F, j, :, :]), s_frf[:])
                    nc.gpsimd.dma_start(orl[F : 2 * F, j, :, W - PAD : W], p2[j])

    nc.compile()
    return nc


def _get_program():
    if "nc" not in _cache:
        _cache["nc"] = _build_program()
    return _cache["nc"]


def _host_prep(fl, fr, variant=None):
    """Build the 8 per-core input maps. fl/fr: [F, H, W] f32 contiguous."""
    variant = variant or VARIANT
    if variant == "H6":
        # H6: packed [flp|frp], [wid|wid], [flpre|frsuf]
        def pack_h(x):
            Fv, Hv, Wv = x.shape
            return np.ascontiguousarray(x.reshape(Fv * HH, HL * Wv))

        zc = np.zeros((F, HPC, PAD), dtype=np.float32)
        widr = np.tile(np.arange(PAD, dtype=np.float32), (F * HH, 2 * HL))
        in_maps = []
        for c in range(NCORES):
            h0 = HPC * c
            flc = np.ascontiguousarray(fl[:, h0 : h0 + HPC, :])
            frc = np.ascontiguousarray(fr[:, h0 : h0 + HPC, :])
            in_maps.append(
                {
                    "flp": np.concatenate(
                        [
                            pack_h(np.concatenate([flc, zc], axis=2)),
                            pack_h(np.concatenate([zc, frc], axis=2)),
                        ],
                        axis=1,
                    ),
                    "wid": widr,
                    "pre": np.concatenate(
                        [
                            pack_h(np.ascontiguousarray(flc[:, :, 0:PAD])),
                            pack_h(
                                np.ascontiguousarray(frc[:, :, W - PAD : W])
                            ),
                        ],
                        axis=1,
                    ),
                }
            )
        return in_maps

    if variant == "H4":
        def pack_h(x):
            Fv, Hv, Wv = x.shape
            return np.ascontiguousarray(x.reshape(Fv * HH, HL * Wv))

        def pack_w(x):
            # [F, HPC, W'] -> w-major [F, W'*HPC] (free = (w, h))
            Fv, Hv, Wv = x.shape
            return np.ascontiguousarray(
                np.transpose(x, (0, 2, 1)).reshape(Fv, Wv * Hv)
            )

        zc = np.zeros((F, HPC, PAD), dtype=np.float32)
        widr = np.tile(np.arange(PAD, dtype=np.float32), (F * HH, HL))
        in_maps = []
        for c in range(NCORES):
            h0 = HPC * c
            flc = np.ascontiguousarray(fl[:, h0 : h0 + HPC, :])
            frc = np.ascontiguousarray(fr[:, h0 : h0 + HPC, :])
            in_maps.append(
                {
                    "fla": pack_h(flc),
                    "frb": pack_h(frc),
                    "flw": pack_w(np.concatenate([flc, zc], axis=2)),
                    "frw": pack_w(np.concatenate([zc, frc], axis=2)),
                    "flpre": pack_h(np.ascontiguousarray(flc[:, :, 0:PAD])),
                    "frsuf": pack_h(
                        np.ascontiguousarray(frc[:, :, W - PAD : W])
                    ),
                    "wid": widr,
                }
            )
        return in_maps

    if variant in ("H2", "H3"):
        # H2/H3: only the two padded windows + the column-index ramp.
        def pack(x):
            Fv, Hv, Wv = x.shape
            return np.ascontiguousarray(x.reshape(Fv * HH, HL * Wv))

        zc = np.zeros((F, HPC, PAD), dtype=np.float32)
        widr = np.tile(np.arange(PAD, dtype=np.float32), (F * HH, HL))
        in_maps = []
        for c in range(NCORES):
            h0 = HPC * c
            flc = np.ascontiguousarray(fl[:, h0 : h0 + HPC, :])
            frc = np.ascontiguousarray(fr[:, h0 : h0 + HPC, :])
            in_maps.append(
                {
                    "flp": pack(np.concatenate([flc, zc], axis=2)),
                    "frp": pack(np.concatenate([zc, frc], axis=2)),
                    "wid": widr,
                }
            )
        return in_maps

    if variant == "H":
        # H-sharding: core c gets rows [16c, 16c+16) and all 48 bins.
        # pack: [F, 16, W'] -> [F*4, 4*W'] (partition = f*4 + h_hi)
        def pack(x):
            Fv, Hv, Wv = x.shape
            return np.ascontiguousarray(x.reshape(Fv * HH, HL * Wv))

        zc = np.zeros((F, HPC, PAD), dtype=np.float32)
        widr = np.tile(np.arange(PAD, dtype=np.float32), (F * HH, HL))
        in_maps = []
        for c in range(NCORES):
            h0 = HPC * c
            flc = np.ascontiguousarray(fl[:, h0 : h0 + HPC, :])
            frc = np.ascontiguousarray(fr[:, h0 : h0 + HPC, :])
            in_maps.append(
                {
                    "fla": pack(flc),
                    "frb": pack(frc),
                    "flp": pack(np.concatenate([flc, zc], axis=2)),
                    "frp": pack(np.concatenate([zc, frc], axis=2)),
                    "flpre": pack(np.ascontiguousarray(flc[:, :, 0:PAD])),
                    "frsuf": pack(np.ascontiguousarray(frc[:, :, W - PAD : W])),
                    "wid": widr,
                }
            )
        return in_maps

    z = np.zeros((F, H, PAD), dtype=np.float32)
    fl_pad = np.concatenate([fl, z], axis=2)  # [F, H, 304]
    fr_pad = np.concatenate([z, fr], axis=2)  # [F, H, 304]

    def h_layout(x):
        # [F, H, width] -> [H, F*width]  (partition = h)
        Fv, Hv, Wv = x.shape
        return np.ascontiguousarray(np.transpose(x, (1, 0, 2)).reshape(Hv, Fv * Wv))

    def af_layout(x):
        # [F, H, width] -> [4*F, 32*width]  (partition = h_hi*F + f)
        Fv, Hv, Wv = x.shape
        hl = Hv // 4
        return np.ascontiguousarray(
            np.transpose(x.reshape(Fv, 4, hl, Wv), (1, 0, 2, 3)).reshape(
                4 * Fv, hl * Wv
            )
        )

    if variant == "B":
        to_sbuf_layout = af_layout
        extra = {"flf": af_layout(fl), "frf": af_layout(fr)}
    elif variant == "C":
        to_sbuf_layout = h_layout
        extra = {"flf": af_layout(fl), "frf": af_layout(fr)}
    elif variant == "D":
        to_sbuf_layout = h_layout
        extra = {
            "flpre": h_layout(np.ascontiguousarray(fl[:, :, 0:PAD])),
            "frsuf": h_layout(np.ascontiguousarray(fr[:, :, W - PAD : W])),
            "wid": np.tile(
                np.arange(PAD, dtype=np.float32), (H, F)
            ),  # [H, F*PAD]
        }
    else:
        to_sbuf_layout = h_layout
        extra = {
            "fl48": h_layout(np.ascontiguousarray(fl[:, :, PAD:W])),
            "fr208": h_layout(np.ascontiguousarray(fr[:, :, 0 : W - PAD])),
        }

    w48 = np.arange(PAD)  # mask index for strips
    in_maps = []
    for c in range(NCORES):
        base = DPC * c
        flp = to_sbuf_layout(fl_pad[:, :, base : base + WIN])
        frp = to_sbuf_layout(fr_pad[:, :, 43 - base : 43 - base + WIN])
        ds = base + np.arange(DPC)  # [6]
        if variant == "D":
            # thresholds per partition: [d_0..d_5, 48-d_0..48-d_5]
            # strip masks: keep fl col w  iff w >= d_j;
            #              keep fr col 208+k iff k < 48-d_j
            row = np.concatenate([ds, PAD - ds]).astype(np.float32)
            in_maps.append(
                {
                    "flp": flp,
                    "frp": frp,
                    "thr": np.ascontiguousarray(np.tile(row, (H, 1))),
                    **extra,
                }
            )
            continue
        # p1[j,f,h,w] = fl[f,h,w] if w >= d_j else 0    (w in [0,48))
        m1 = (w48[None, :] >= ds[:, None])[:, None, None, :]  # [6,1,1,48]
        p1 = np.ascontiguousarray(
            np.where(m1, fl[None, :, :, 0:PAD], np.float32(0.0)), dtype=np.float32
        )
        # p2[j,f,h,k] = fr[f,h,208+k] if 208+k < W-d_j else 0
        m2 = ((W - PAD + w48)[None, :] < (W - ds)[:, None])[:, None, None, :]
        p2 = np.ascontiguousarray(
            np.where(m2, fr[None, :, :, W - PAD : W], np.float32(0.0)),
            dtype=np.float32,
        )
        in_maps.append({"flp": flp, "frp": frp, "p1": p1, "p2": p2, **extra})
    return in_maps


def _get_exec():
    """Build (once) a persistent jitted SPMD executor for the bass program.

    Modeled on concourse.bass2jax.run_bass_via_pjrt, but cached so repeat
    calls don't re-trace/re-compile, and without output-buffer donation so
    the same callable can be invoked repeatedly (timing loops).
    """
    if "exec" in _cache:
        return _cache["exec"]

    import jax
    import concourse.mybir as mybir
    from jax.sharding import Mesh, PartitionSpec
    from jax.experimental.shard_map import shard_map
    from concourse.bass2jax import (
        _bass_exec_p,
        install_neuronx_cc_hook,
        partition_id_tensor,
    )

    nc = _get_program()
    install_neuronx_cc_hook()

    partition_name = (
        nc.partition_id_tensor.name if nc.partition_id_tensor else None
    )
    in_names, out_names, out_avals = [], [], []
    for alloc in nc.m.functions[0].allocations:
        if not isinstance(alloc, mybir.MemoryLocationSet):
            continue
        name = alloc.memorylocations[0].name
        if alloc.kind == "ExternalInput":
            if name != partition_name:
                in_names.append(name)
        elif alloc.kind == "ExternalOutput":
            out_names.append(name)
            out_avals.append(
                jax.core.ShapedArray(
                    tuple(alloc.tensor_shape), mybir.dt.np(alloc.dtype)
                )
            )
    n_params = len(in_names)
    all_names = in_names + out_names
    if partition_name is not None:
        all_names = all_names + [partition_name]

    def _body(*args):
        operands = list(args)
        if partition_name is not None:
            operands.append(partition_id_tensor())
        outs = _bass_exec_p.bind(
            *operands,
            out_avals=tuple(out_avals),
            in_names=tuple(all_names),
            out_names=tuple(out_names),
            lowering_input_output_aliases=(),
            sim_require_finite=True,
            sim_require_nnan=True,
            nc=nc,
        )
        return tuple(outs)

    devices = jax.devices()[:NCORES]
    mesh = Mesh(np.asarray(devices), ("core",))
    nin = n_params + len(out_names)
    sharded = jax.jit(
        shard_map(
            _body,
            mesh=mesh,
            in_specs=(PartitionSpec("core"),) * nin,
            out_specs=(PartitionSpec("core"),) * len(out_names),
            check_rep=False,
        ),
        keep_unused=True,
    )
    zeros = [
        np.zeros((NCORES * a.shape[0], *a.shape[1:]), a.dtype) for a in out_avals
    ]
    _cache["exec"] = (sharded, in_names, out_names, out_avals, zeros)
    return _cache["exec"]


def _run(features_left, features_right, bins, trace=False):
    fl = np.ascontiguousarray(np.asarray(features_left, dtype=np.float32)[0])
    fr = np.ascontiguousarray(np.asarray(features_right, dtype=np.float32)[0])
    in_maps = _host_prep(fl, fr)
    sharded, in_names, out_names, out_avals, zeros = _get_exec()
    concat_in = [
        np.concatenate([in_maps[c][name] for c in range(NCORES)], axis=0)
        for name in in_names
    ]
    out_arrs = sharded(*concat_in, *zeros)
    outs = {
        name: np.asarray(out_arrs[i]).reshape(NCORES, *out_avals[i].shape)
        for i, name in enumerate(out_names)
    }
    vol_lr = np.empty((B, 2 * F, D, H, W), dtype=np.float32)
    vol_rl = np.empty((B, 2 * F, D, H, W), dtype=np.float32)
    for c in range(NCORES):
        if VARIANT.startswith("H"):
            vol_lr[0, :, :, HPC * c : HPC * (c + 1)] = outs["olr"][c]
            vol_rl[0, :, :, HPC * c : HPC * (c + 1)] = outs["orl"][c]
        else:
            vol_lr[0, :, DPC * c : DPC * (c + 1)] = outs["olr"][c]
            vol_rl[0, :, DPC * c : DPC * (c + 1)] = outs["orl"][c]
    return (vol_lr, vol_rl), None


def _reference_np(features_left, features_right, bins):
    """Numpy fallback for unexpected bins (kept for robustness)."""
    fl = np.asarray(features_left, dtype=np.float32)
    fr = np.asarray(features_right, dtype=np.float32)
    bins = np.asarray(bins)
    Bv, Fv, Hv, Wv = fl.shape
    w = np.arange(Wv)
    b = bins[:, None]
    idx_m = np.clip(w[None, :] - b, 0, Wv - 1)
    idx_p = np.clip(w[None, :] + b, 0, Wv - 1)
    m_lr = (w[None, :] >= b)[None, None, :, None, :]
    m_rl = (w[None, :] < Wv - b)[None, None, :, None, :]
    g_r = np.transpose(fr[:, :, :, idx_m], (0, 1, 3, 2, 4))
    g_l = np.transpose(fl[:, :, :, idx_p], (0, 1, 3, 2, 4))
    bl = fl[:, :, None, :, :]
    br = fr[:, :, None, :, :]
    zero = np.float32(0.0)
    vol_lr = np.concatenate(
        [np.where(m_lr, bl, zero), np.where(m_lr, g_r, zero)], axis=1
    )
    vol_rl = np.concatenate(
        [np.where(m_rl, g_l, zero), np.where(m_rl, br, zero)], axis=1
    )
    return vol_lr.astype(np.float32), vol_rl.astype(np.float32)


def kernel(features_left, features_right, bins):
    fl = np.asarray(features_left)
    fr = np.asarray(features_right)
    b = np.asarray(bins)
    if (
        fl.shape != (B, F, H, W)
        or fr.shape != (B, F, H, W)
        or b.shape != (D,)
        or not np.array_equal(b, np.arange(D))
    ):
        return _reference_np(features_left, features_right, bins)
    out, _ = _run(fl, fr, b, trace=False)
    return out

